# revision 1
# baseline (speedup 1.0000x reference)
"""Trainium2 Bass kernel for nn_ExpertDistillationLoss.

Strategy (data-parallel over batch, 8 cores, 1 batch element each):
  - Device (per core): the FLOP-heavy expert-MSE pipeline.
      d.T[h, s] = W_s·sh.T − W_t·th.T  (bf16 operands, f32 PSUM accumulation,
      host-pre-transposed weight/activation layouts, W stationary)
      mean_base via d² accumulation + per-chunk ones-matmuls,
      cross+quad terms fused into one PSUM accumulator V[s, 256] built from
      (a) P-matmuls of d.T tiles against a host-prescaled B_cat and
      (b) Gram-matrix matmuls against host-precomputed G pairs,
      then one broadcasted DVE multiply/reduce against a_s/a_t.
      Device output per core: feat partial = Σ wsel·mse (1 scalar)
      (+ small debug tensors).
  - Host: input sharding/layout, the K=3 MC sampling scan (gates-only, exact
    argmax semantics), method-B losses, and the final scalar combine.
"""

import numpy as np
import ml_dtypes

B, S, H, E, R, K = 8, 2048, 2048, 8, 16, 3
ALPHA = 0.5
LAMBDA_COV = 0.5
BETA_ENT = 0.1
TEMP_LO, TEMP_HI = 0.5, 1.5
SCALE_T = 2.0
SCALE_S = 2.0
EPS = 1e-8

NK = H // 128          # 16 k-tiles
NM = H // 128          # 16 output h-tiles
NNS = 4                # s-chunks of 512
NSUB = 4               # 128-token subchunks per s-chunk
NCHUNK = S // 128      # 16

BF16 = ml_dtypes.bfloat16

_PROGRAM_CACHE = {}


# ----------------------------------------------------------------------------
# device program
# ----------------------------------------------------------------------------

def _build_program(db_nonzero: bool, debug_out: bool = False):
    import concourse.bacc as bacc
    import concourse.tile as tile
    from concourse import mybir

    f32 = mybir.dt.float32
    bf16 = mybir.dt.bfloat16
    ALU = mybir.AluOpType
    AX = mybir.AxisListType

    kt = NK + (1 if db_nonzero else 0)   # extra k-tile carries the bias row

    nc = bacc.Bacc("TRN2", target_bir_lowering=False, debug=False)

    # DRAM inputs (per-core shapes; layouts are host-prepared)
    d_shT = nc.dram_tensor("shT", [128, kt, S], bf16, kind="ExternalInput").ap()
    d_thT = nc.dram_tensor("thT", [128, NK, S], bf16, kind="ExternalInput").ap()
    d_Ws = nc.dram_tensor("Ws", [NM, 128, kt, 128], bf16, kind="ExternalInput").ap()
    d_Wt = nc.dram_tensor("Wt", [NM, 128, NK, 128], bf16, kind="ExternalInput").ap()
    d_Bc = nc.dram_tensor("Bcat", [128, NM, 256], bf16, kind="ExternalInput").ap()
    d_Gs = nc.dram_tensor("Gs", [16, 256], bf16, kind="ExternalInput").ap()
    d_Gt = nc.dram_tensor("Gt", [16, 256], bf16, kind="ExternalInput").ap()
    d_acat = nc.dram_tensor("acat", [128, NCHUNK, 32], f32, kind="ExternalInput").ap()
    d_asT = nc.dram_tensor("asT", [16, S], bf16, kind="ExternalInput").ap()
    d_atT = nc.dram_tensor("atT", [16, S], bf16, kind="ExternalInput").ap()
    d_wsel = nc.dram_tensor("wsel", [128, 128], f32, kind="ExternalInput").ap()
    d_wsele = nc.dram_tensor("wsel_e", [128, 16], f32, kind="ExternalInput").ap()
    d_onesH = nc.dram_tensor("onesH", [128, 1], f32, kind="ExternalInput").ap()
    d_ones1 = nc.dram_tensor("ones1", [128, 1], f32, kind="ExternalInput").ap()

    # outputs
    d_feat = nc.dram_tensor("feat", [1, 1], f32, kind="ExternalOutput").ap()
    if debug_out:
        d_msed = nc.dram_tensor("mse_dbg", [128, 128], f32, kind="ExternalOutput").ap()
        d_mbd = nc.dram_tensor("mb_dbg", [128, 16], f32, kind="ExternalOutput").ap()
        d_dtd = nc.dram_tensor("dt_dbg", [NM, 128, 512], bf16, kind="ExternalOutput").ap()
        d_accd = nc.dram_tensor("acc_dbg", [128, S], f32, kind="ExternalOutput").ap()

    with tile.TileContext(nc) as tc:
        with (
            tc.tile_pool(name="const", bufs=1) as cp,
            tc.tile_pool(name="wst", bufs=6) as wp,
            tc.tile_pool(name="dT", bufs=2) as dp,
            tc.tile_pool(name="sq", bufs=2) as qp,
            tc.tile_pool(name="vc", bufs=2) as vp,
        ):
            from contextlib import ExitStack
            _mp = ExitStack()
            pd = _mp.enter_context(tc.tile_pool(name="pd", bufs=3, space="PSUM"))
            pv = _mp.enter_context(tc.tile_pool(name="pv", bufs=5, space="PSUM"))
            # ---- resident loads ----
            # DMA emission order matters for startup: the first m-tiles' W
            # stripes and the first s-chunk's activation slices go first so
            # PE can start ~15us in instead of waiting for the bulk load.
            NPRE = 3
            whead = []
            for m in range(NPRE):
                ws0 = wp.tile([128, kt * 128], bf16, tag="w", name=f"wsh_{m}")
                nc.sync.dma_start(ws0[:], d_Ws[m].rearrange("p a b -> p (a b)"))
                wt0 = wp.tile([128, NK * 128], bf16, tag="w", name=f"wth_{m}")
                nc.sync.dma_start(wt0[:], d_Wt[m].rearrange("p a b -> p (a b)"))
                whead.append((ws0, wt0))

            shT = cp.tile([128, kt * S], bf16, tag="shT")
            thT = cp.tile([128, NK * S], bf16, tag="thT")
            for c0, c1 in ((0, 1024), (1024, S)):
                for k in range(kt):
                    nc.sync.dma_start(shT[:, k * S + c0:k * S + c1],
                                      d_shT[:, k, c0:c1])
                    if k < NK:
                        nc.sync.dma_start(thT[:, k * S + c0:k * S + c1],
                                          d_thT[:, k, c0:c1])
            Bc = cp.tile([128, NM * 256], bf16, tag="Bc")
            nc.sync.dma_start(Bc[:], d_Bc[:].rearrange("p a b -> p (a b)"))
            Gs = cp.tile([16, 256], bf16, tag="Gs")
            nc.sync.dma_start(Gs[:], d_Gs)
            Gt = cp.tile([16, 256], bf16, tag="Gt")
            nc.sync.dma_start(Gt[:], d_Gt)
            acat_sb = cp.tile([128, NCHUNK * 32], f32, tag="acat")
            nc.sync.dma_start(acat_sb[:], d_acat[:].rearrange("p a b -> p (a b)"))
            asT_sb = cp.tile([16, S], bf16, tag="asT")
            nc.sync.dma_start(asT_sb[:], d_asT)
            atT_sb = cp.tile([16, S], bf16, tag="atT")
            nc.sync.dma_start(atT_sb[:], d_atT)
            wsel = cp.tile([128, 128], f32, tag="wsel")
            nc.sync.dma_start(wsel[:], d_wsel)
            wsele = cp.tile([128, 16], f32, tag="wsele")
            nc.sync.dma_start(wsele[:], d_wsele)
            onesH = cp.tile([128, 1], f32, tag="onesH")
            nc.sync.dma_start(onesH[:], d_onesH)
            ones1 = cp.tile([128, 1], f32, tag="ones1")
            nc.sync.dma_start(ones1[:], d_ones1)

            acc128 = cp.tile([128, S], f32, tag="acc128")
            nc.vector.memset(acc128[:], 0.0)
            mse_sb = cp.tile([128, 128], f32, tag="mse")
            mb_sb = cp.tile([128, 16], f32, tag="mb")

            # ---- main loop: s-chunk pairs sharing one W load ----
            # dTc caches the second chunk's d tiles so its P-matmuls (and the
            # 4-bank V accumulation) run after the first chunk's V is consumed.
            dTc = cp.tile([128, NM * 512], bf16, tag="dTc")

            def consume_v(Vt, base_chunk):
                for sub in range(NSUB):
                    chunk = base_chunk + sub
                    ab = acat_sb[:, chunk * 32:(chunk + 1) * 32].rearrange(
                        "p (t r) -> p t r", t=2)
                    ab = ab.unsqueeze(2).broadcast_to([128, 2, 8, 16])
                    prod = vp.tile([128, 256], f32, tag="prod",
                                   name=f"prod_{chunk}")
                    nc.vector.tensor_tensor(
                        prod[:].rearrange("p (t e r) -> p t e r", t=2, e=8),
                        Vt[sub][:].rearrange("p (t e r) -> p t e r", t=2, e=8),
                        ab, ALU.mult)
                    red = vp.tile([128, 16], f32, tag="red", name=f"red_{chunk}")
                    nc.vector.tensor_reduce(
                        red[:], prod[:].rearrange("p (t e r) -> p t e r", t=2, e=8),
                        axis=AX.X, op=ALU.add)
                    nc.vector.tensor_add(mse_sb[:, chunk * 8:(chunk + 1) * 8],
                                         red[:, 0:8], red[:, 8:16])

            def u_mms(Vt, s0):
                for sub in range(NSUB):
                    t0 = s0 + sub * 128
                    nc.tensor.matmul(Vt[sub][:], asT_sb[:, t0:t0 + 128],
                                     Gs[:], start=True, stop=False)
                    nc.tensor.matmul(Vt[sub][:], atT_sb[:, t0:t0 + 128],
                                     Gt[:], start=False, stop=False)

            for nsp in range(NNS // 2):
                s0a = nsp * 1024
                s0b = s0a + 512
                Va = [pv.tile([128, 256], f32, tag="V", name=f"Va_{nsp}_{j}")
                      for j in range(NSUB)]
                for m in range(NM):
                    if nsp == 0 and m < NPRE:
                        ws, wt = whead[m]
                    else:
                        ws = wp.tile([128, kt * 128], bf16, tag="w",
                                     name=f"ws_{nsp}_{m}")
                        wsf = d_Ws[m].rearrange("p a b -> p (a b)")
                        hw_ = (kt * 128) // 2
                        nc.sync.dma_start(ws[:, 0:hw_], wsf[:, 0:hw_])
                        nc.sync.dma_start(ws[:, hw_:kt * 128], wsf[:, hw_:kt * 128])
                        wt = wp.tile([128, NK * 128], bf16, tag="w",
                                     name=f"wt_{nsp}_{m}")
                        wtf = d_Wt[m].rearrange("p a b -> p (a b)")
                        nc.sync.dma_start(wt[:, 0:NK * 64], wtf[:, 0:NK * 64])
                        nc.sync.dma_start(wt[:, NK * 64:NK * 128], wtf[:, NK * 64:NK * 128])

                    pds = []
                    for half, s0 in ((0, s0a), (1, s0b)):
                        pd_t = pd.tile([128, 512], f32, tag="pd",
                                       name=f"pd_{nsp}_{m}_{half}")
                        pds.append(pd_t)
                        for k in range(kt):
                            rs = shT[:, k * S + s0: k * S + s0 + 512]
                            nc.tensor.matmul(pd_t[:], ws[:, k * 128:(k + 1) * 128],
                                             rs, start=(k == 0),
                                             stop=(k == kt - 1 and kt > NK))
                            if k < NK:
                                rt = thT[:, k * S + s0: k * S + s0 + 512]
                                nc.tensor.matmul(pd_t[:], wt[:, k * 128:(k + 1) * 128],
                                                 rt, start=False,
                                                 stop=(k == NK - 1 and kt == NK))
                        if half == 0:
                            # dT copy runs on ACT while PE streams half1's
                            # k-loop, so the P-matmuls below don't stall PE
                            dT = dp.tile([128, 512], bf16, tag="dT",
                                         name=f"dT_{nsp}_{m}")
                            nc.scalar.copy(dT[:], pds[0][:])
                            if debug_out and nsp == 0:
                                nc.sync.dma_start(d_dtd[m], dT[:])
                            if m == 0:
                                u_mms(Va, s0a)

                    for half, s0 in ((0, s0a), (1, s0b)):
                        pd_t = pds[half]
                        sq = qp.tile([128, 512], f32, tag="sq",
                                     name=f"sq_{nsp}_{m}_{half}")
                        nc.scalar.square(sq[:], pd_t[:])
                        nc.vector.tensor_add(acc128[:, s0:s0 + 512],
                                             acc128[:, s0:s0 + 512], sq[:])
                        if half == 0:
                            for sub in range(NSUB):
                                nc.tensor.matmul(Va[sub][:],
                                                 dT[:, sub * 128:(sub + 1) * 128],
                                                 Bc[:, m * 256:(m + 1) * 256],
                                                 start=False, stop=(m == NM - 1))
                        else:
                            nc.scalar.copy(dTc[:, m * 512:(m + 1) * 512], pd_t[:])

                consume_v(Va, nsp * NSUB * 2)

                Vb = [pv.tile([128, 256], f32, tag="V", name=f"Vb_{nsp}_{j}")
                      for j in range(NSUB)]
                u_mms(Vb, s0b)
                for m in range(NM):
                    for sub in range(NSUB):
                        nc.tensor.matmul(Vb[sub][:],
                                         dTc[:, m * 512 + sub * 128: m * 512 + (sub + 1) * 128],
                                         Bc[:, m * 256:(m + 1) * 256],
                                         start=False, stop=(m == NM - 1))
                consume_v(Vb, nsp * NSUB * 2 + NSUB)

            # ---- mean_base: per-chunk ones-matmuls ----
            _mp.close()
            pm_ctx = tc.tile_pool(name="pm", bufs=1, space="PSUM")
            pm = pm_ctx.__enter__()
            mbp = pm.tile([128, 512], f32, tag="pmisc")
            for c in range(NCHUNK):
                nc.tensor.matmul(mbp[:, c:c + 1], acc128[:, c * 128:(c + 1) * 128],
                                 onesH[:], start=True, stop=True)
            nc.scalar.copy(mb_sb[:], mbp[:, 0:16])

            # ---- feat partial ----
            scr1 = cp.tile([128, 128], f32, tag="scr1")
            fx = cp.tile([128, 1], f32, tag="fx")
            nc.vector.tensor_mul(scr1[:], mse_sb[:], wsel[:])
            nc.vector.tensor_reduce(fx[:], scr1[:], axis=AX.X, op=ALU.add)
            scr2 = cp.tile([128, 16], f32, tag="scr2")
            fmb = cp.tile([128, 1], f32, tag="fmb")
            nc.vector.tensor_mul(scr2[:], mb_sb[:], wsele[:])
            nc.vector.tensor_reduce(fmb[:], scr2[:], axis=AX.X, op=ALU.add)
            fsum = cp.tile([128, 1], f32, tag="fsum")
            nc.vector.tensor_add(fsum[:], fx[:], fmb[:])
            fp = pm.tile([128, 512], f32, tag="pmisc")
            nc.tensor.matmul(fp[0:1, 0:1], fsum[:], ones1[:], start=True, stop=True)
            fout = cp.tile([1, 1], f32, tag="fout")
            nc.scalar.copy(fout[:], fp[0:1, 0:1])

            pm_ctx.__exit__(None, None, None)
            nc.sync.dma_start(d_feat, fout[:])
            if debug_out:
                nc.sync.dma_start(d_msed, mse_sb[:])
                nc.sync.dma_start(d_mbd, mb_sb[:])
                nc.sync.dma_start(d_accd, acc128[:])

    nc.compile()
    return nc


def _get_program(db_nonzero: bool, debug_out: bool = False):
    key = (bool(db_nonzero), bool(debug_out))
    if key not in _PROGRAM_CACHE:
        _PROGRAM_CACHE[key] = _build_program(*key)
    return _PROGRAM_CACHE[key]


# ----------------------------------------------------------------------------
# host side
# ----------------------------------------------------------------------------

def _host_scan_all(tg_all, sg_all, mask_f, gumbel):
    """Method-A sampling scan, all cores vectorized. Exact argmax semantics.
    Returns (wsel[B,S,E] f32, wsum f64, t_counts[E] f64, s_counts[E] f64)."""
    f32 = np.float32
    p = tg_all.astype(f32).copy()
    wsel = np.zeros((B, S, E), f32)
    BIG = f32(1e4)
    iota = np.arange(E, dtype=f32)
    for k in range(K):
        z = np.log(p) + gumbel[k]
        m = z.max(-1, keepdims=True)
        ge = (z >= m).astype(f32)
        t = iota + BIG - BIG * ge
        idxf = t.min(-1, keepdims=True)
        oh = (iota == idxf).astype(f32)
        po = p * oh
        w = po.sum(-1)
        sg_k = (sg_all * oh).sum(-1)
        mw = mask_f * w
        wsel += mw[..., None] * oh
        if k < K - 1:
            pn = p + (ALPHA - 1.0) * po
            p = pn / pn.sum(-1, keepdims=True)
    # counts from wsel (mw·oh summed over k) and the student-gate variant
    t_counts = wsel.astype(np.float64).sum(axis=(0, 1))
    wsum = float(t_counts.sum())
    # recompute s-side accumulation (needs per-step oh); cheap second pass
    p = tg_all.astype(f32).copy()
    s_counts = np.zeros(E, np.float64)
    for k in range(K):
        z = np.log(p) + gumbel[k]
        m = z.max(-1, keepdims=True)
        ge = (z >= m).astype(f32)
        t = iota + BIG - BIG * ge
        idxf = t.min(-1, keepdims=True)
        oh = (iota == idxf).astype(f32)
        po = p * oh
        sg_k = (sg_all * oh).sum(-1)
        s_counts += ((mask_f * sg_k)[..., None] * oh).astype(np.float64).sum(axis=(0, 1))
        if k < K - 1:
            pn = p + (ALPHA - 1.0) * po
            p = pn / pn.sum(-1, keepdims=True)
    return wsel, wsum, t_counts, s_counts


def _host_method_b(tg, sg, temp_c):
    """Per-core method-B partials: (tkl, ent)."""
    f32 = np.float32
    tg = tg.astype(f32)
    sg = sg.astype(f32)
    sgT = sg / f32(temp_c)
    ltg = np.log(tg)
    lsg = np.log(sg)
    ent = (sg * lsg).sum(dtype=f32)
    mb2 = sgT.max(-1, keepdims=True)
    ex = np.exp(sgT - mb2)
    se = ex.sum(-1, keepdims=True, dtype=f32)
    lse = np.log(se) + mb2
    sum_tg = tg.sum(-1, keepdims=True, dtype=f32)
    tkl = (tg * (ltg - sgT)).sum(dtype=f32) + (lse * sum_tg).sum(dtype=f32)
    return tkl, ent


def _prep_shared(inputs, db_nonzero):
    """Replicated (per-core identical) device arrays."""
    f32 = np.float32
    W_t = np.asarray(inputs["W_t"], f32)
    W_s = np.asarray(inputs["W_s"], f32)
    A_t = np.asarray(inputs["A_t"], f32)
    A_s = np.asarray(inputs["A_s"], f32)
    B_t = np.asarray(inputs["B_t"], f32)
    B_s = np.asarray(inputs["B_s"], f32)
    db = (np.asarray(inputs["b_s"], f32) - np.asarray(inputs["b_t"], f32))

    kt = NK + (1 if db_nonzero else 0)

    # W layout [m, p, k, c] = W[m*128+c, k*128+p]
    def w_host(W, k_tiles, bias=None):
        out = np.zeros((NM, 128, k_tiles, 128), BF16)
        out[:, :, :NK, :] = (
            W.astype(BF16).reshape(NM, 128, NK, 128).transpose(0, 3, 2, 1)
        )
        if bias is not None and k_tiles > NK:
            # bias block: partition 0 row carries db[m*128+c]
            out[:, 0, NK, :] = bias.astype(BF16).reshape(NM, 128)
        return np.ascontiguousarray(out)

    Ws = w_host(W_s, kt, db if db_nonzero else None)
    Wt = w_host(-W_t, NK)   # negated: PSUM accumulation adds, d = base_s - base_t


    # Bcat [p, m, 256]
    Bs_her = B_s.transpose(1, 0, 2).reshape(H, E * R)
    Bt_her = B_t.transpose(1, 0, 2).reshape(H, E * R)
    B_cat = np.concatenate(
        [(2.0 * SCALE_S / H) * Bs_her, (-2.0 * SCALE_T / H) * Bt_her], axis=1
    ).astype(BF16)
    Bcat = np.ascontiguousarray(B_cat.reshape(NM, 128, 256).transpose(1, 0, 2))

    # Gram pairs [16, 256]
    G_ss = np.einsum("ehr,ehq->erq", B_s, B_s)
    G_st = np.einsum("ehr,ehq->erq", B_s, B_t)
    G_tt = np.einsum("ehr,ehq->erq", B_t, B_t)
    G_stT = G_st.transpose(0, 2, 1)

    def to_req(G):
        return G.transpose(1, 0, 2).reshape(R, E * R)

    Gs = np.concatenate(
        [(SCALE_S * SCALE_S / H) * to_req(G_ss),
         (-SCALE_S * SCALE_T / H) * to_req(G_st)], axis=1).astype(BF16)
    Gt = np.concatenate(
        [(-SCALE_S * SCALE_T / H) * to_req(G_stT),
         (SCALE_T * SCALE_T / H) * to_req(G_tt)], axis=1).astype(BF16)

    onesH = np.full((128, 1), 1.0 / H, f32)
    ones1 = np.ones((128, 1), f32)

    shared = dict(Ws=Ws, Wt=Wt, Bcat=Bcat, Gs=Gs, Gt=Gt,
                  onesH=onesH, ones1=ones1)
    mats = dict(A_sT=np.ascontiguousarray(A_s.T), A_tT=np.ascontiguousarray(A_t.T))
    return shared, mats, kt


def _prep_core(inputs, core, kt, wsel, mats):
    """Per-core device arrays."""
    f32 = np.float32
    sh = np.asarray(inputs["student_hidden_states"][core], f32)
    th = np.asarray(inputs["teacher_hidden_states"][core], f32)

    a_s = sh @ mats["A_sT"]                      # [S, R] f32
    a_t = th @ mats["A_tT"]
    acat = np.concatenate([a_s, a_t], axis=1)    # [S, 32]
    acat = np.ascontiguousarray(
        acat.reshape(NCHUNK, 128, 32).transpose(1, 0, 2)).astype(f32)
    asT = np.ascontiguousarray(a_s.T).astype(BF16)
    atT = np.ascontiguousarray(a_t.T).astype(BF16)

    # [p, k, s] layout of x.T (k = inner dim of x)
    def xt_host(x, k_tiles, ones_tail=False):
        out = np.zeros((128, k_tiles, S), BF16)
        out[:, :NK, :] = x.T.astype(BF16).reshape(NK, 128, S).transpose(1, 0, 2)
        if ones_tail and k_tiles > NK:
            out[0, NK, :] = BF16(1.0)
        return np.ascontiguousarray(out)

    shT = xt_host(sh, kt, ones_tail=(kt > NK))
    thT = xt_host(th, NK)

    wsel_dev = np.ascontiguousarray(
        wsel.reshape(NCHUNK, 128, E).transpose(1, 0, 2).reshape(128, 128)).astype(f32)
    wsel_e = np.ascontiguousarray(wsel.sum(-1).reshape(NCHUNK, 128).T).astype(f32)
    return dict(shT=shT, thT=thT, wsel=wsel_dev, wsel_e=wsel_e,
                acat=acat, asT=asT, atT=atT)


def _combine(feat_parts, wsum, t_counts, s_counts, tkls, ents, temp_c):
    f32 = np.float32
    feat = np.sum(np.asarray(feat_parts, f32), dtype=f32)
    tc = np.asarray(t_counts, np.float64)
    sc = np.asarray(s_counts, np.float64)
    tkl = np.sum(np.asarray(tkls, f32), dtype=f32)
    ent = np.sum(np.asarray(ents, f32), dtype=f32)

    feat_loss = feat / max(wsum, 1e-8)
    t_avg = tc / tc.sum() + EPS
    s_avg = sc / sc.sum() + EPS
    t_avg = t_avg / t_avg.sum()
    s_avg = s_avg / s_avg.sum()
    coverage_kl = (t_avg * (np.log(t_avg) - np.log(s_avg))).sum() / E
    method_a_total = feat_loss + LAMBDA_COV * coverage_kl
    temp_kl = tkl / B
    entropy_loss = ent / (B * S)
    method_b_total = temp_kl + BETA_ENT * entropy_loss
    return np.array(
        [feat_loss, coverage_kl, method_a_total, temp_kl, entropy_loss,
         method_b_total, temp_c], f32)


def _host_all(inputs):
    """Host scan/method-B for all cores + per-core device input maps."""
    f32 = np.float32
    db_nonzero = bool(
        np.any(np.asarray(inputs["b_s"], f32) != np.asarray(inputs["b_t"], f32)))
    temp = float(np.asarray(inputs["temperature"], f32))
    temp_c = float(np.clip(temp, TEMP_LO, TEMP_HI))

    u = np.asarray(inputs["uniform_noise"], f32)
    gumbel = -np.log(-np.log(u * (1.0 - 2e-7) + 1e-7)).astype(f32)
    mask_f = np.asarray(inputs["attention_mask"], f32)
    tg_all = np.asarray(inputs["teacher_gates"], f32)
    sg_all = np.asarray(inputs["student_gates"], f32)

    shared, mats, kt = _prep_shared(inputs, db_nonzero)
    wsel_all, wsum, t_counts, s_counts = _host_scan_all(
        tg_all, sg_all, mask_f, gumbel)

    in_maps = []
    tkls, ents = [], []
    for c in range(B):
        tkl, ent = _host_method_b(tg_all[c], sg_all[c], temp_c)
        tkls.append(tkl)
        ents.append(ent)
        m = dict(shared)
        m.update(_prep_core(inputs, c, kt, wsel_all[c], mats))
        in_maps.append(m)

    return dict(in_maps=in_maps, db_nonzero=db_nonzero, temp_c=temp_c,
                wsum=wsum, t_counts=t_counts, s_counts=s_counts,
                tkls=tkls, ents=ents)


def kernel(**inputs) -> np.ndarray:
    host = _host_all(inputs)
    nc = _get_program(host["db_nonzero"])

    from concourse.bass_utils import run_bass_kernel_spmd

    res = run_bass_kernel_spmd(nc, host["in_maps"], core_ids=list(range(B)))
    feat_parts = [float(res.results[c]["feat"][0, 0]) for c in range(B)]

    return _combine(feat_parts, host["wsum"], host["t_counts"],
                    host["s_counts"], host["tkls"], host["ents"],
                    host["temp_c"])



# revision 3
# speedup vs baseline: 3.7541x; 3.7541x over previous
"""Trainium2 Bass kernel for nn_ExpertDistillationLoss — fp8 DoubleRow version.

Strategy (data-parallel over batch, 8 cores, 1 batch element each):
  - Device (per core): the FLOP-heavy expert-MSE pipeline in fp8e4 DoubleRow
    (2 fp8 weights/cell, 256-deep contraction per matmul, 0.5 cycles/row):
      pd[c, t] = 64*(sh@Ws.T - th@Wt.T) via 16 DR matmuls per (m, chunk),
      weights fully resident in SBUF (fp8, 64 KB/partition total).
      mean_base: ACT square of pd -> DVE accumulate -> per-chunk ones-matmuls.
      cross: ACT downcast dT8 = pd/4 (fp8) -> DR P-matmuls against resident
      Bc8 (m-tile pairs) into V[t, 256] -> DVE multiply/reduce vs host-scaled
      acat. Device output per core: feat partial (mean_base+cross) scalar.
  - Host: input sharding + fp8 layout prep, the K=3 MC sampling scan (exact
    argmax semantics), the quad (LoRA Gram) term at sampled experts,
    method-B losses, and the final scalar combine.
"""

import numpy as np
import ml_dtypes

B, S, H, E, R, K = 8, 2048, 2048, 8, 16, 3
ALPHA = 0.5
LAMBDA_COV = 0.5
BETA_ENT = 0.1
TEMP_LO, TEMP_HI = 0.5, 1.5
SCALE_T = 2.0
SCALE_S = 2.0
EPS = 1e-8

NK2 = H // 256         # 8 k2-tiles (256-deep DoubleRow contraction)
NM = H // 128          # 16 output h-tiles
NMP = NM // 2          # 8 m-tile pairs for the P-matmul
NC = S // 512          # 4 s-chunks of 512
NSUB = 4               # 128-token subchunks per s-chunk
NCHUNK = S // 128      # 16

WSC = 64.0             # weight prescale (power of 2; keeps fp8 W out of subnormals)
DSC = 0.25             # dT8 = DSC * pd  (pd = WSC*d, so dT8 = 16*d)
BSC = 256.0            # B prescale for fp8
VSC = (WSC * DSC) * BSC    # 4096: scale carried by V = dT8^T Bc8

import os
N_WARM = int(os.environ.get("KV2_WARM", "40"))   # PE clock-warmup dummies
N_WARM2 = int(os.environ.get("KV2_WARM2", "0"))  # second burst (th chunk-0)
P0_FIRST = os.environ.get("KV2_P0FIRST", "0") == "1"

BF16 = ml_dtypes.bfloat16
F8 = ml_dtypes.float8_e4m3

_PROGRAM_CACHE = {}


# ----------------------------------------------------------------------------
# device program
# ----------------------------------------------------------------------------

def _build_program(db_nonzero: bool):
    import os
    import concourse.bacc as bacc
    import concourse.tile as tile
    from concourse import mybir

    f32 = mybir.dt.float32
    fp8 = mybir.dt.float8e4
    ALU = mybir.AluOpType
    AX = mybir.AxisListType
    DR = mybir.MatmulPerfMode.DoubleRow

    nke = NK2 + (1 if db_nonzero else 0)   # extra k2-tile carries the bias row

    nc = bacc.Bacc("TRN2", target_bir_lowering=False, debug=False)

    # DRAM inputs (per-core shapes; layouts are host-prepared)
    d_sh = nc.dram_tensor("sh8", [128, nke, 2, S], fp8, kind="ExternalInput").ap()
    d_th = nc.dram_tensor("th8", [128, NK2, 2, S], fp8, kind="ExternalInput").ap()
    d_Ws = nc.dram_tensor("Ws8", [128, NM, nke, 2, 128], fp8, kind="ExternalInput").ap()
    d_Wt = nc.dram_tensor("Wt8", [128, NM, NK2, 2, 128], fp8, kind="ExternalInput").ap()
    d_Bc = nc.dram_tensor("Bc8", [128, NMP, 2, 256], fp8, kind="ExternalInput").ap()
    bf16 = mybir.dt.bfloat16
    d_wa = nc.dram_tensor("wacat", [128, NCHUNK, 256], bf16, kind="ExternalInput").ap()
    d_onesH = nc.dram_tensor("onesH", [128, 1], f32, kind="ExternalInput").ap()

    d_fm = nc.dram_tensor("fm", [128, 32], f32, kind="ExternalOutput").ap()

    with tile.TileContext(nc) as tc:
        with (
            tc.tile_pool(name="const", bufs=1) as cp,
            tc.tile_pool(name="sq", bufs=2) as qp,
            tc.tile_pool(name="vc", bufs=2) as vp,
        ):
            from contextlib import ExitStack
            _mp = ExitStack()
            pd = _mp.enter_context(tc.tile_pool(name="pd", bufs=int(__import__("os").environ.get("KV2_PDB","3")), space="PSUM"))
            pv = _mp.enter_context(tc.tile_pool(name="pv", bufs=int(__import__("os").environ.get("KV2_PVB","4")), space="PSUM"))
            pm = _mp.enter_context(tc.tile_pool(name="pm", bufs=1, space="PSUM"))

            # ---- PE clock warmup: dummy DR matmuls on memset tiles run
            # while the first real DMAs are in flight, so the p-state ramp
            # completes before the first real k-loop.
            dwu = cp.tile([128, 256], fp8, tag="dwu")
            nc.vector.memset(dwu[:], 0.0)
            dpd = pd.tile([128, 128], f32, tag="pd", name="warm_pd")
            dwv = dwu[:].rearrange("p (j c) -> p j c", j=2)
            if N_WARM:
                for i in range(N_WARM):
                    nc.tensor.matmul(dpd[:], dwv, dwv,
                                     start=(i == 0), stop=(i == N_WARM - 1),
                                     perf_mode=DR)

            def warm2():
                if not N_WARM2:
                    return
                # pv pool: its banks are untouched until the first P-block,
                # so this group cannot collide with the open pd k-loop groups
                dpd2 = pv.tile([128, 128], f32, tag="V", name="warm_pd2")
                for i in range(N_WARM2):
                    nc.tensor.matmul(dpd2[:], dwv, dwv,
                                     start=(i == 0), stop=(i == N_WARM2 - 1),
                                     perf_mode=DR)

            # ---- resident loads; emission order = DMA service order ----
            Ws = cp.tile([128, NM * nke * 2 * 128], fp8, tag="Ws")
            Wt = cp.tile([128, NM * NK2 * 2 * 128], fp8, tag="Wt")
            sh = cp.tile([128, nke * 2 * S], fp8, tag="sh")
            th = cp.tile([128, NK2 * 2 * S], fp8, tag="th")
            wstr = nke * 2 * 128       # Ws stride per m
            wttr = NK2 * 2 * 128       # Wt stride per m
            d_Wsf = d_Ws[:].rearrange("p m k j c -> p (m k j c)")
            d_Wtf = d_Wt[:].rearrange("p m k j c -> p (m k j c)")
            shv = sh[:].rearrange("p (k j s) -> p k j s", k=nke, j=2)
            thv = th[:].rearrange("p (k j s) -> p k j s", k=NK2, j=2)

            def load_w(m):
                nc.sync.dma_start(Ws[:, m * wstr:(m + 1) * wstr],
                                  d_Wsf[:, m * wstr:(m + 1) * wstr])
                nc.sync.dma_start(Wt[:, m * wttr:(m + 1) * wttr],
                                  d_Wtf[:, m * wttr:(m + 1) * wttr])

            def load_act(c):
                s0, s1 = c * 512, (c + 1) * 512
                nc.sync.dma_start(shv[:, :, :, s0:s1], d_sh[:, :, :, s0:s1])
                nc.sync.dma_start(thv[:, :, :, s0:s1], d_th[:, :, :, s0:s1])

            nc.sync.dma_start(Ws[:, 0:wstr], d_Wsf[:, 0:wstr])
            nc.sync.dma_start(shv[:, :, :, 0:512], d_sh[:, :, :, 0:512])
            nc.sync.dma_start(Wt[:, 0:wttr], d_Wtf[:, 0:wttr])
            nc.sync.dma_start(thv[:, :, :, 0:512], d_th[:, :, :, 0:512])
            wa_sb = cp.tile([128, NCHUNK * 256], bf16, tag="wa")

            def load_wa(c):
                nc.sync.dma_start(
                    wa_sb[:, c * 1024:(c + 1) * 1024],
                    d_wa[:].rearrange("p a b -> p (a b)")[:, c * 1024:(c + 1) * 1024])

            # straight weight stream: supply (1.46us/m-pair) stays ahead of
            # the steady k-loop demand (1.71us/m), so no mid-chunk stalls;
            # Bc and the chunk-1 acts land just before P(0)/k(1,0) need them
            for m in range(1, NM):
                load_w(m)
            Bc = cp.tile([128, NMP * 2 * 256], fp8, tag="Bc")
            nc.sync.dma_start(Bc[:], d_Bc[:].rearrange("p a b c -> p (a b c)"))
            load_act(1)
            load_act(2)
            load_act(3)
            for c in range(NC):
                load_wa(c)
            onesH = cp.tile([128, 1], f32, tag="onesH")
            nc.sync.dma_start(onesH[:], d_onesH)

            acc128 = cp.tile([128, S], f32, tag="acc128")
            nc.vector.memset(acc128[:], 0.0)
            fm = cp.tile([128, 32], f32, tag="fm")   # [fmse(16) | mb(16)]
            mbp = pm.tile([128, 16], f32, tag="pmisc")

            dTc0 = cp.tile([128, NM * 512], fp8, tag="dTc0")
            dTc1 = cp.tile([128, NM * 512], fp8, tag="dTc1")

            Wsv = Ws[:].rearrange("p (m k j c) -> p m k j c", m=NM, k=nke, j=2)
            Wtv = Wt[:].rearrange("p (m k j c) -> p m k j c", m=NM, k=NK2, j=2)

            CHUNKS = [(c * 512, 512) for c in range(NC)]
            NCH = len(CHUNKS)

            pds = {}

            def k_s(c, m):
                s0, w = CHUNKS[c]
                pd_t = pd.tile([128, w], f32, tag="pd", name=f"pd_{c}_{m}")
                pds[(c, m)] = pd_t
                for k2 in range(nke):
                    nc.tensor.matmul(pd_t[:], Wsv[:, m, k2],
                                     shv[:, k2, :, s0:s0 + w],
                                     start=(k2 == 0), stop=False, perf_mode=DR)

            def k_t(c, m, dTc):
                s0, w = CHUNKS[c]
                pd_t = pds.pop((c, m))
                for k2 in range(NK2):
                    nc.tensor.matmul(pd_t[:], Wtv[:, m, k2],
                                     thv[:, k2, :, s0:s0 + w],
                                     start=False, stop=(k2 == NK2 - 1),
                                     perf_mode=DR)
                nc.scalar.mul(dTc[:, m * 512:m * 512 + w], pd_t[:], DSC)
                sq = qp.tile([128, w], f32, tag="sq", name=f"sq_{c}_{m}")
                nc.scalar.square(sq[:], pd_t[:])
                add_eng = nc.gpsimd if os.environ.get("KV2_ADDE", "dve") == "gps" \
                    else nc.vector
                add_eng.tensor_add(acc128[:, s0:s0 + w],
                                   acc128[:, s0:s0 + w], sq[:])

            def k_loop(c, m, dTc):
                k_s(c, m)
                k_t(c, m, dTc)

            def consume_v(Vt, chunk, sub):
                # wacat already carries wsel * a * scale: one multiply + one
                # full-width reduce per 128-token chunk. Multiplies alternate
                # DVE / GpSimd and reduces DVE / ACT-accumulator, spreading
                # the final chunk's drain across three engines.
                prod = vp.tile([128, 256], f32, tag="prod",
                               name=f"prod_{chunk}")
                nc.vector.tensor_tensor(prod[:], Vt[:],
                                        wa_sb[:, chunk * 256:(chunk + 1) * 256],
                                        ALU.mult)
                if sub in (0, 1, 2):
                    # ACT accumulator takes one reduce off DVE; the LAST
                    # sub's reduce stays on DVE so the tail's final hop
                    # avoids a cross-engine semaphore
                    nc.scalar.activation(
                        prod[:], prod[:], mybir.ActivationFunctionType.Copy,
                        accum_out=fm[:, chunk:chunk + 1])
                else:
                    nc.vector.tensor_reduce(fm[:, chunk:chunk + 1], prod[:],
                                            axis=AX.X, op=ALU.add)

            # P-block split: the head (m-pairs 0..5) depends only on dT8
            # tiles the ACT queue finished long ago; the rest (pairs 6, 7 +
            # consume) waits on the final dT8 copies and is emitted one
            # k-loop later so PE never idles on the ACT lag.
            def p_head(c, dTc):
                nsub = CHUNKS[c][1] // 128
                dv = dTc[:].rearrange("p (m s) -> p m s", m=NM)
                Bv = Bc[:].rearrange("p (a b c) -> p a b c", a=NMP, b=2)
                Vts = []
                for sub in range(nsub):
                    t0 = sub * 128
                    Vt = pv.tile([128, 256], f32, tag="V", name=f"V_{c}_{sub}")
                    Vts.append(Vt)
                    for m2 in range(NMP - 2):
                        nc.tensor.matmul(
                            Vt[:],
                            dv[:, 2 * m2:2 * m2 + 2, t0:t0 + 128],
                            Bv[:, m2],
                            start=(m2 == 0), stop=False,
                            perf_mode=DR)
                return Vts

            def p_rest(c, dTc, Vts):
                s0, w = CHUNKS[c]
                dv = dTc[:].rearrange("p (m s) -> p m s", m=NM)
                Bv = Bc[:].rearrange("p (a b c) -> p a b c", a=NMP, b=2)
                for sub in range(w // 128):
                    t0 = sub * 128
                    for m2 in (NMP - 2, NMP - 1):
                        nc.tensor.matmul(
                            Vts[sub][:],
                            dv[:, 2 * m2:2 * m2 + 2, t0:t0 + 128],
                            Bv[:, m2],
                            start=False, stop=(m2 == NMP - 1),
                            perf_mode=DR)
                    consume_v(Vts[sub], s0 // 128 + sub, sub)

            def p_block(c, dTc):
                p_rest(c, dTc, p_head(c, dTc))

            def mb_mms(c):
                # per-128-token-chunk ones-matmuls over this chunk's acc128
                s0, w = CHUNKS[c]
                for i in range(w // 128):
                    cc = s0 // 128 + i
                    nc.tensor.matmul(mbp[:, cc:cc + 1],
                                     acc128[:, cc * 128:(cc + 1) * 128],
                                     onesH[:], start=True, stop=True)

            # ---- software-pipelined main loop ----
            # chunk 0 runs the s-matmuls 3 m-tiles ahead of the t-matmuls so
            # PE need not wait for the th chunk-0 DMA; later chunks pair s/t.
            # P-matmuls of chunk c are emitted after k(c+1, m=0) so they never
            # wait on the in-flight ACT dT8 copies.
            dTcs = [dTc0, dTc1]
            SKEW = int(os.environ.get("KV2_SKEW", "1"))
            for m in range(NM + SKEW):
                if m < NM:
                    k_s(0, m)
                if m == SKEW - 1:
                    warm2()
                if m >= SKEW:
                    k_t(0, m - SKEW, dTcs[0])
            # P(0) before k(1,0): chunk-1 activations are still in flight
            # when chunk 0 ends, so the P-block fills that DMA wait. For
            # later chunks the acts are long resident and P runs after
            # k(c,0) to hide the dT8 ACT lag instead.
            if P0_FIRST:
                p_block(0, dTcs[0])
                mb_mms(0)
            pend = None
            for c in range(1, NCH):
                for m in range(NM):
                    k_loop(c, m, dTcs[c % 2])
                    if m == 0 and (c > 1 or not P0_FIRST):
                        pend = p_head(c - 1, dTcs[(c - 1) % 2])
                    if m == 1 and pend is not None:
                        p_rest(c - 1, dTcs[(c - 1) % 2], pend)
                        mb_mms(c - 1)
                        pend = None
            p_block(NCH - 1, dTcs[(NCH - 1) % 2])
            mb_mms(NCH - 1)
            nc.scalar.copy(fm[:, 16:32], mbp[:, 0:16])
            _mp.close()
            nc.sync.dma_start(d_fm, fm[:])

    nc.compile()
    return nc


def _get_program(db_nonzero: bool):
    key = bool(db_nonzero)
    if key not in _PROGRAM_CACHE:
        _PROGRAM_CACHE[key] = _build_program(key)
    return _PROGRAM_CACHE[key]


# ----------------------------------------------------------------------------
# host side
# ----------------------------------------------------------------------------

def _host_scan_all(tg_all, sg_all, mask_f, gumbel):
    """Method-A sampling scan, all cores vectorized. Exact argmax semantics.
    Returns (wsel[B,S,E] f32, wsum f64, t_counts[E] f64, s_counts[E] f64)."""
    f32 = np.float32
    p = tg_all.astype(f32).copy()
    wsel = np.zeros((B, S, E), f32)
    s_counts = np.zeros(E, np.float64)
    BIG = f32(1e4)
    iota = np.arange(E, dtype=f32)
    for k in range(K):
        z = np.log(p) + gumbel[k]
        m = z.max(-1, keepdims=True)
        ge = (z >= m).astype(f32)
        t = iota + BIG - BIG * ge
        idxf = t.min(-1, keepdims=True)
        oh = (iota == idxf).astype(f32)
        po = p * oh
        w = po.sum(-1)
        sg_k = (sg_all * oh).sum(-1)
        mw = mask_f * w
        wsel += mw[..., None] * oh
        s_counts += ((mask_f * sg_k)[..., None] * oh).astype(np.float64).sum(axis=(0, 1))
        if k < K - 1:
            pn = p + (ALPHA - 1.0) * po
            p = pn / pn.sum(-1, keepdims=True)
    t_counts = wsel.astype(np.float64).sum(axis=(0, 1))
    wsum = float(t_counts.sum())
    return wsel, wsum, t_counts, s_counts


def _host_method_b(tg, sg, temp_c):
    """Per-core method-B partials: (tkl, ent)."""
    f32 = np.float32
    tg = tg.astype(f32)
    sg = sg.astype(f32)
    sgT = sg / f32(temp_c)
    ltg = np.log(tg)
    lsg = np.log(sg)
    ent = (sg * lsg).sum(dtype=f32)
    mb2 = sgT.max(-1, keepdims=True)
    ex = np.exp(sgT - mb2)
    se = ex.sum(-1, keepdims=True, dtype=f32)
    lse = np.log(se) + mb2
    sum_tg = tg.sum(-1, keepdims=True, dtype=f32)
    tkl = (tg * (ltg - sgT)).sum(dtype=f32) + (lse * sum_tg).sum(dtype=f32)
    return tkl, ent


def _host_quad(a_s_all, a_t_all, wsel, B_s, B_t):
    """Sum over tokens/experts of wsel * quad / H (the LoRA Gram term)."""
    G_ss = np.einsum("ehr,ehq->erq", B_s, B_s)
    G_st = np.einsum("ehr,ehq->erq", B_s, B_t)
    G_tt = np.einsum("ehr,ehq->erq", B_t, B_t)
    acc = 0.0
    for e in range(E):
        q1 = ((a_s_all @ G_ss[e]) * a_s_all).sum(-1)
        q2 = ((a_s_all @ G_st[e]) * a_t_all).sum(-1)
        q3 = ((a_t_all @ G_tt[e]) * a_t_all).sum(-1)
        qe = (SCALE_S * SCALE_S) * q1 - (2.0 * SCALE_S * SCALE_T) * q2 \
            + (SCALE_T * SCALE_T) * q3
        acc += float((wsel[:, :, e].astype(np.float64) * qe).sum())
    return acc / H


def _prep_shared(inputs, db_nonzero):
    """Replicated (per-core identical) device arrays."""
    f32 = np.float32
    W_t = np.asarray(inputs["W_t"], f32)
    W_s = np.asarray(inputs["W_s"], f32)
    B_t = np.asarray(inputs["B_t"], f32)
    B_s = np.asarray(inputs["B_s"], f32)
    db = (np.asarray(inputs["b_s"], f32) - np.asarray(inputs["b_t"], f32))

    nke = NK2 + (1 if db_nonzero else 0)

    # W layout [p, m, k2, j, c] = WSC * W[m*128+c, k2*256+j*128+p]
    def w_host(W, k_tiles, bias=None):
        out = np.zeros((128, NM, k_tiles, 2, 128), F8)
        out[:, :, :NK2] = (WSC * W).astype(F8).reshape(
            NM, 128, NK2, 2, 128).transpose(4, 0, 2, 3, 1)
        if bias is not None and k_tiles > NK2:
            out[0, :, NK2, 0, :] = (WSC * bias).astype(F8).reshape(NM, 128)
        return np.ascontiguousarray(out)

    Ws8 = w_host(W_s, nke, db if db_nonzero else None)
    Wt8 = w_host(-W_t, NK2)   # negated: PSUM accumulation adds, d = base_s - base_t

    # Bc8 [p, m2, j, col] = BSC * [Bs_her | Bt_her][m2*256+j*128+p, col]
    Bs_her = B_s.transpose(1, 0, 2).reshape(H, E * R)
    Bt_her = B_t.transpose(1, 0, 2).reshape(H, E * R)
    B_cat = np.concatenate([Bs_her, Bt_her], axis=1)
    Bc8 = np.ascontiguousarray(
        (BSC * B_cat).astype(F8).reshape(NMP, 2, 128, 256).transpose(2, 0, 1, 3))

    onesH = np.full((128, 1), 1.0 / (H * WSC * WSC), f32)

    shared = dict(Ws8=Ws8, Wt8=Wt8, Bc8=Bc8, onesH=onesH)
    return shared, nke


def _prep_core(sh, th, a_s, a_t, nke, wsel, db_nonzero):
    """Per-core device arrays (plus host-side wsel_e for the mb combine)."""
    f32 = np.float32

    # wacat[t, (half, e, r)] = wsel[t, e] * f_half * a_half[t, r]: folds the
    # expert selection weights and cross-term scales into the V consume.
    fs = f32(2.0 * SCALE_S / (H * VSC))
    ft = f32(-2.0 * SCALE_T / (H * VSC))
    wa_s = wsel[:, :, None] * (fs * a_s)[:, None, :]       # [S, E, R]
    wa_t = wsel[:, :, None] * (ft * a_t)[:, None, :]
    wa = np.concatenate([wa_s.reshape(S, E * R), wa_t.reshape(S, E * R)],
                        axis=1)                            # [S, 256]
    wacat = np.ascontiguousarray(
        wa.reshape(NCHUNK, 128, 256).transpose(1, 0, 2)).astype(BF16)

    # [p, k2, j, s] layout of x.T
    def xt_host(x, k_tiles, ones_tail=False):
        out = np.zeros((128, k_tiles, 2, S), F8)
        out[:, :NK2] = x.T.astype(F8).reshape(NK2, 2, 128, S).transpose(2, 0, 1, 3)
        if ones_tail and k_tiles > NK2:
            out[0, NK2, 0, :] = F8(1.0)
        return np.ascontiguousarray(out)

    sh8 = xt_host(sh, nke, ones_tail=(nke > NK2))
    th8 = xt_host(th, NK2)

    wsel_e = np.ascontiguousarray(
        wsel.sum(-1).reshape(NCHUNK, 128).T).astype(f32)   # [128, NCHUNK]
    dev = dict(sh8=sh8, th8=th8, wacat=wacat)
    return dev, wsel_e


def _combine(feat_parts, feat_quad, wsum, t_counts, s_counts, tkls, ents, temp_c):
    f32 = np.float32
    feat = np.sum(np.asarray(feat_parts, f32), dtype=f32) + f32(feat_quad)
    tc = np.asarray(t_counts, np.float64)
    sc = np.asarray(s_counts, np.float64)
    tkl = np.sum(np.asarray(tkls, f32), dtype=f32)
    ent = np.sum(np.asarray(ents, f32), dtype=f32)

    feat_loss = feat / max(wsum, 1e-8)
    t_avg = tc / tc.sum() + EPS
    s_avg = sc / sc.sum() + EPS
    t_avg = t_avg / t_avg.sum()
    s_avg = s_avg / s_avg.sum()
    coverage_kl = (t_avg * (np.log(t_avg) - np.log(s_avg))).sum() / E
    method_a_total = feat_loss + LAMBDA_COV * coverage_kl
    temp_kl = tkl / B
    entropy_loss = ent / (B * S)
    method_b_total = temp_kl + BETA_ENT * entropy_loss
    return np.array(
        [feat_loss, coverage_kl, method_a_total, temp_kl, entropy_loss,
         method_b_total, temp_c], f32)


def _host_all(inputs):
    """Host scan/method-B/quad for all cores + per-core device input maps."""
    f32 = np.float32
    db_nonzero = bool(
        np.any(np.asarray(inputs["b_s"], f32) != np.asarray(inputs["b_t"], f32)))
    temp = float(np.asarray(inputs["temperature"], f32))
    temp_c = float(np.clip(temp, TEMP_LO, TEMP_HI))

    u = np.asarray(inputs["uniform_noise"], f32)
    gumbel = -np.log(-np.log(u * (1.0 - 2e-7) + 1e-7)).astype(f32)
    mask_f = np.asarray(inputs["attention_mask"], f32)
    tg_all = np.asarray(inputs["teacher_gates"], f32)
    sg_all = np.asarray(inputs["student_gates"], f32)

    shared, nke = _prep_shared(inputs, db_nonzero)
    wsel_all, wsum, t_counts, s_counts = _host_scan_all(
        tg_all, sg_all, mask_f, gumbel)

    A_sT = np.ascontiguousarray(np.asarray(inputs["A_s"], f32).T)
    A_tT = np.ascontiguousarray(np.asarray(inputs["A_t"], f32).T)
    sh_all = np.asarray(inputs["student_hidden_states"], f32)
    th_all = np.asarray(inputs["teacher_hidden_states"], f32)

    in_maps = []
    tkls, ents, wsel_es = [], [], []
    a_s_all = np.empty((B, S, R), f32)
    a_t_all = np.empty((B, S, R), f32)
    for c in range(B):
        tkl, ent = _host_method_b(tg_all[c], sg_all[c], temp_c)
        tkls.append(tkl)
        ents.append(ent)
        a_s = sh_all[c] @ A_sT
        a_t = th_all[c] @ A_tT
        a_s_all[c] = a_s
        a_t_all[c] = a_t
        m = dict(shared)
        dev, wsel_e = _prep_core(sh_all[c], th_all[c], a_s, a_t, nke,
                                 wsel_all[c], db_nonzero)
        m.update(dev)
        in_maps.append(m)
        wsel_es.append(wsel_e)

    feat_quad = _host_quad(a_s_all, a_t_all, wsel_all,
                           np.asarray(inputs["B_s"], f32),
                           np.asarray(inputs["B_t"], f32))

    return dict(in_maps=in_maps, db_nonzero=db_nonzero, temp_c=temp_c,
                wsum=wsum, t_counts=t_counts, s_counts=s_counts,
                tkls=tkls, ents=ents, feat_quad=feat_quad, wsel_es=wsel_es)


def kernel(**inputs) -> np.ndarray:
    host = _host_all(inputs)
    nc = _get_program(host["db_nonzero"])

    from concourse.bass_utils import run_bass_kernel_spmd

    res = run_bass_kernel_spmd(nc, host["in_maps"], core_ids=list(range(B)))
    feat_parts = []
    for c in range(B):
        fm = np.asarray(res.results[c]["fm"], np.float32)   # [128, 32]
        fmse = fm[:, 0:16]
        mb = fm[:, 16:32]
        feat_parts.append(float(fmse.sum(dtype=np.float64))
                          + float((mb * host["wsel_es"][c]).sum(dtype=np.float64)))

    return _combine(feat_parts, host["feat_quad"], host["wsum"],
                    host["t_counts"], host["s_counts"], host["tkls"],
                    host["ents"], host["temp_c"])


# revision 4
# speedup vs baseline: 3.7606x; 1.0017x over previous
"""Trainium2 Bass kernel for nn_ExpertDistillationLoss — fp8 DoubleRow version.

Strategy (data-parallel over batch, 8 cores, 1 batch element each):
  - Device (per core): the FLOP-heavy expert-MSE pipeline in fp8e4 DoubleRow
    (2 fp8 weights/cell, 256-deep contraction per matmul, 0.5 cycles/row):
      pd[c, t] = 64*(sh@Ws.T - th@Wt.T) via 16 DR matmuls per (m, chunk),
      weights fully resident in SBUF (fp8, 64 KB/partition total).
      mean_base: ACT square of pd -> DVE accumulate -> per-chunk ones-matmuls.
      cross: ACT downcast dT8 = pd/4 (fp8) -> DR P-matmuls against resident
      Bc8 (m-tile pairs) into V[t, 256] -> DVE multiply/reduce vs host-scaled
      acat. Device output per core: feat partial (mean_base+cross) scalar.
  - Host: input sharding + fp8 layout prep, the K=3 MC sampling scan (exact
    argmax semantics), the quad (LoRA Gram) term at sampled experts,
    method-B losses, and the final scalar combine.
"""

import numpy as np
import ml_dtypes

B, S, H, E, R, K = 8, 2048, 2048, 8, 16, 3
ALPHA = 0.5
LAMBDA_COV = 0.5
BETA_ENT = 0.1
TEMP_LO, TEMP_HI = 0.5, 1.5
SCALE_T = 2.0
SCALE_S = 2.0
EPS = 1e-8

NK2 = H // 256         # 8 k2-tiles (256-deep DoubleRow contraction)
NM = H // 128          # 16 output h-tiles
NMP = NM // 2          # 8 m-tile pairs for the P-matmul
NC = S // 512          # 4 s-chunks of 512
NSUB = 4               # 128-token subchunks per s-chunk
NCHUNK = S // 128      # 16

WSC = 64.0             # weight prescale (power of 2; keeps fp8 W out of subnormals)
DSC = 0.25             # dT8 = DSC * pd  (pd = WSC*d, so dT8 = 16*d)
BSC = 256.0            # B prescale for fp8
VSC = (WSC * DSC) * BSC    # 4096: scale carried by V = dT8^T Bc8

import os
N_WARM = int(os.environ.get("KV2_WARM", "40"))   # PE clock-warmup dummies
N_WARM2 = int(os.environ.get("KV2_WARM2", "0"))  # second burst (th chunk-0)
P0_FIRST = os.environ.get("KV2_P0FIRST", "0") == "1"

BF16 = ml_dtypes.bfloat16
F8 = ml_dtypes.float8_e4m3

_PROGRAM_CACHE = {}


# ----------------------------------------------------------------------------
# device program
# ----------------------------------------------------------------------------

def _build_program(db_nonzero: bool):
    import os
    import concourse.bacc as bacc
    import concourse.tile as tile
    from concourse import mybir

    f32 = mybir.dt.float32
    fp8 = mybir.dt.float8e4
    ALU = mybir.AluOpType
    AX = mybir.AxisListType
    DR = mybir.MatmulPerfMode.DoubleRow

    nke = NK2 + (1 if db_nonzero else 0)   # extra k2-tile carries the bias row

    nc = bacc.Bacc("TRN2", target_bir_lowering=False, debug=False)

    # DRAM inputs (per-core shapes; layouts are host-prepared)
    d_sh = nc.dram_tensor("sh8", [128, nke, 2, S], fp8, kind="ExternalInput").ap()
    d_th = nc.dram_tensor("th8", [128, NK2, 2, S], fp8, kind="ExternalInput").ap()
    d_Ws = nc.dram_tensor("Ws8", [128, NM, nke, 2, 128], fp8, kind="ExternalInput").ap()
    d_Wt = nc.dram_tensor("Wt8", [128, NM, NK2, 2, 128], fp8, kind="ExternalInput").ap()
    d_Bc = nc.dram_tensor("Bc8", [128, NMP, 2, 256], fp8, kind="ExternalInput").ap()
    bf16 = mybir.dt.bfloat16
    d_wa = nc.dram_tensor("wacat", [128, NCHUNK, 256], bf16, kind="ExternalInput").ap()
    d_onesH = nc.dram_tensor("onesH", [128, 1], bf16, kind="ExternalInput").ap()

    d_fm = nc.dram_tensor("fm", [128, 32], f32, kind="ExternalOutput").ap()

    with tile.TileContext(nc) as tc:
        with (
            tc.tile_pool(name="const", bufs=1) as cp,
            tc.tile_pool(name="sq", bufs=2) as qp,
            tc.tile_pool(name="vc", bufs=2) as vp,
        ):
            from contextlib import ExitStack
            _mp = ExitStack()
            pd = _mp.enter_context(tc.tile_pool(name="pd", bufs=int(__import__("os").environ.get("KV2_PDB","3")), space="PSUM"))
            pv = _mp.enter_context(tc.tile_pool(name="pv", bufs=int(__import__("os").environ.get("KV2_PVB","4")), space="PSUM"))
            pm = _mp.enter_context(tc.tile_pool(name="pm", bufs=1, space="PSUM"))

            # ---- PE clock warmup: dummy DR matmuls on memset tiles run
            # while the first real DMAs are in flight, so the p-state ramp
            # completes before the first real k-loop.
            dwu = cp.tile([128, 256], fp8, tag="dwu")
            nc.vector.memset(dwu[:], 0.0)
            dpd = pd.tile([128, 128], f32, tag="pd", name="warm_pd")
            dwv = dwu[:].rearrange("p (j c) -> p j c", j=2)
            if N_WARM:
                for i in range(N_WARM):
                    nc.tensor.matmul(dpd[:], dwv, dwv,
                                     start=(i == 0), stop=(i == N_WARM - 1),
                                     perf_mode=DR)

            def warm2():
                if not N_WARM2:
                    return
                # pv pool: its banks are untouched until the first P-block,
                # so this group cannot collide with the open pd k-loop groups
                dpd2 = pv.tile([128, 128], f32, tag="V", name="warm_pd2")
                for i in range(N_WARM2):
                    nc.tensor.matmul(dpd2[:], dwv, dwv,
                                     start=(i == 0), stop=(i == N_WARM2 - 1),
                                     perf_mode=DR)

            # ---- resident loads; emission order = DMA service order ----
            Ws = cp.tile([128, NM * nke * 2 * 128], fp8, tag="Ws")
            Wt = cp.tile([128, NM * NK2 * 2 * 128], fp8, tag="Wt")
            sh = cp.tile([128, nke * 2 * S], fp8, tag="sh")
            th = cp.tile([128, NK2 * 2 * S], fp8, tag="th")
            wstr = nke * 2 * 128       # Ws stride per m
            wttr = NK2 * 2 * 128       # Wt stride per m
            d_Wsf = d_Ws[:].rearrange("p m k j c -> p (m k j c)")
            d_Wtf = d_Wt[:].rearrange("p m k j c -> p (m k j c)")
            shv = sh[:].rearrange("p (k j s) -> p k j s", k=nke, j=2)
            thv = th[:].rearrange("p (k j s) -> p k j s", k=NK2, j=2)

            def load_w(m):
                nc.sync.dma_start(Ws[:, m * wstr:(m + 1) * wstr],
                                  d_Wsf[:, m * wstr:(m + 1) * wstr])
                nc.sync.dma_start(Wt[:, m * wttr:(m + 1) * wttr],
                                  d_Wtf[:, m * wttr:(m + 1) * wttr])

            def load_act(c):
                s0, s1 = c * 512, (c + 1) * 512
                nc.sync.dma_start(shv[:, :, :, s0:s1], d_sh[:, :, :, s0:s1])
                nc.sync.dma_start(thv[:, :, :, s0:s1], d_th[:, :, :, s0:s1])

            head_eng = nc.gpsimd if os.environ.get("KV2_HEADDMA", "sync") == "gps" \
                else nc.sync
            head_eng.dma_start(Ws[:, 0:wstr], d_Wsf[:, 0:wstr])
            head_eng.dma_start(shv[:, :, :, 0:512], d_sh[:, :, :, 0:512])
            head_eng.dma_start(Wt[:, 0:wttr], d_Wtf[:, 0:wttr])
            head_eng.dma_start(thv[:, :, :, 0:512], d_th[:, :, :, 0:512])
            wa_sb = cp.tile([128, NCHUNK * 256], bf16, tag="wa")

            def load_wa(c):
                nc.sync.dma_start(
                    wa_sb[:, c * 1024:(c + 1) * 1024],
                    d_wa[:].rearrange("p a b -> p (a b)")[:, c * 1024:(c + 1) * 1024])

            # straight weight stream: supply (1.46us/m-pair) stays ahead of
            # the steady k-loop demand (1.71us/m), so no mid-chunk stalls;
            # Bc and the chunk-1 acts land just before P(0)/k(1,0) need them
            for m in range(1, NM):
                load_w(m)
            Bc = cp.tile([128, NMP * 2 * 256], fp8, tag="Bc")
            nc.sync.dma_start(Bc[:], d_Bc[:].rearrange("p a b c -> p (a b c)"))
            load_act(1)
            load_act(2)
            load_act(3)
            for c in range(NC):
                load_wa(c)
            onesH = cp.tile([128, 1], bf16, tag="onesH")
            nc.sync.dma_start(onesH[:], d_onesH)

            acc128 = cp.tile([128, S], bf16, tag="acc128")
            nc.vector.memset(acc128[:], 0.0)
            fm = cp.tile([128, 32], f32, tag="fm")   # [fmse(16) | mb(16)]
            mbp = pm.tile([128, 16], f32, tag="pmisc")

            dTc0 = cp.tile([128, NM * 512], fp8, tag="dTc0")
            dTc1 = cp.tile([128, NM * 512], fp8, tag="dTc1")

            Wsv = Ws[:].rearrange("p (m k j c) -> p m k j c", m=NM, k=nke, j=2)
            Wtv = Wt[:].rearrange("p (m k j c) -> p m k j c", m=NM, k=NK2, j=2)

            CHUNKS = [(c * 512, 512) for c in range(NC)]
            NCH = len(CHUNKS)

            pds = {}

            def k_s(c, m):
                s0, w = CHUNKS[c]
                pd_t = pd.tile([128, w], f32, tag="pd", name=f"pd_{c}_{m}")
                pds[(c, m)] = pd_t
                for k2 in range(nke):
                    nc.tensor.matmul(pd_t[:], Wsv[:, m, k2],
                                     shv[:, k2, :, s0:s0 + w],
                                     start=(k2 == 0), stop=False, perf_mode=DR)

            def k_t(c, m, dTc):
                s0, w = CHUNKS[c]
                pd_t = pds.pop((c, m))
                for k2 in range(NK2):
                    nc.tensor.matmul(pd_t[:], Wtv[:, m, k2],
                                     thv[:, k2, :, s0:s0 + w],
                                     start=False, stop=(k2 == NK2 - 1),
                                     perf_mode=DR)
                nc.scalar.mul(dTc[:, m * 512:m * 512 + w], pd_t[:], DSC)
                sq = qp.tile([128, w], bf16, tag="sq", name=f"sq_{c}_{m}")
                nc.scalar.square(sq[:], pd_t[:])
                # bf16 accumulate: DVE 2x mode (all-16-bit packed SBUF ops);
                # ~0.3% noise on mean_base, far inside the 2e-2 budget
                with nc.allow_low_precision("bf16 mean-base accumulate"):
                    nc.vector.tensor_add(acc128[:, s0:s0 + w],
                                         acc128[:, s0:s0 + w], sq[:])

            def k_loop(c, m, dTc):
                k_s(c, m)
                k_t(c, m, dTc)

            def consume_v(Vt, chunk, sub):
                # wacat already carries wsel * a * scale: one multiply + one
                # full-width reduce per 128-token chunk. Multiplies alternate
                # DVE / GpSimd and reduces DVE / ACT-accumulator, spreading
                # the final chunk's drain across three engines.
                prod = vp.tile([128, 256], f32, tag="prod",
                               name=f"prod_{chunk}")
                nc.vector.tensor_tensor(prod[:], Vt[:],
                                        wa_sb[:, chunk * 256:(chunk + 1) * 256],
                                        ALU.mult)
                if sub in (0, 1, 2):
                    # ACT accumulator takes one reduce off DVE; the LAST
                    # sub's reduce stays on DVE so the tail's final hop
                    # avoids a cross-engine semaphore
                    nc.scalar.activation(
                        prod[:], prod[:], mybir.ActivationFunctionType.Copy,
                        accum_out=fm[:, chunk:chunk + 1])
                else:
                    nc.vector.tensor_reduce(fm[:, chunk:chunk + 1], prod[:],
                                            axis=AX.X, op=ALU.add)

            # P-block split: the head (m-pairs 0..5) depends only on dT8
            # tiles the ACT queue finished long ago; the rest (pairs 6, 7 +
            # consume) waits on the final dT8 copies and is emitted one
            # k-loop later so PE never idles on the ACT lag.
            def p_head(c, dTc):
                nsub = CHUNKS[c][1] // 128
                dv = dTc[:].rearrange("p (m s) -> p m s", m=NM)
                Bv = Bc[:].rearrange("p (a b c) -> p a b c", a=NMP, b=2)
                Vts = []
                for sub in range(nsub):
                    t0 = sub * 128
                    Vt = pv.tile([128, 256], f32, tag="V", name=f"V_{c}_{sub}")
                    Vts.append(Vt)
                    for m2 in range(NMP - 2):
                        nc.tensor.matmul(
                            Vt[:],
                            dv[:, 2 * m2:2 * m2 + 2, t0:t0 + 128],
                            Bv[:, m2],
                            start=(m2 == 0), stop=False,
                            perf_mode=DR)
                return Vts

            def p_rest(c, dTc, Vts):
                s0, w = CHUNKS[c]
                dv = dTc[:].rearrange("p (m s) -> p m s", m=NM)
                Bv = Bc[:].rearrange("p (a b c) -> p a b c", a=NMP, b=2)
                for sub in range(w // 128):
                    t0 = sub * 128
                    for m2 in (NMP - 2, NMP - 1):
                        nc.tensor.matmul(
                            Vts[sub][:],
                            dv[:, 2 * m2:2 * m2 + 2, t0:t0 + 128],
                            Bv[:, m2],
                            start=False, stop=(m2 == NMP - 1),
                            perf_mode=DR)
                    consume_v(Vts[sub], s0 // 128 + sub, sub)

            def p_block(c, dTc):
                p_rest(c, dTc, p_head(c, dTc))

            def mb_mms(c):
                # per-128-token-chunk ones-matmuls over this chunk's acc128
                s0, w = CHUNKS[c]
                for i in range(w // 128):
                    cc = s0 // 128 + i
                    nc.tensor.matmul(mbp[:, cc:cc + 1],
                                     acc128[:, cc * 128:(cc + 1) * 128],
                                     onesH[:], start=True, stop=True)

            # ---- software-pipelined main loop ----
            # chunk 0 runs the s-matmuls 3 m-tiles ahead of the t-matmuls so
            # PE need not wait for the th chunk-0 DMA; later chunks pair s/t.
            # P-matmuls of chunk c are emitted after k(c+1, m=0) so they never
            # wait on the in-flight ACT dT8 copies.
            dTcs = [dTc0, dTc1]
            SKEW = int(os.environ.get("KV2_SKEW", "1"))
            for m in range(NM + SKEW):
                if m < NM:
                    k_s(0, m)
                if m == SKEW - 1:
                    warm2()
                if m >= SKEW:
                    k_t(0, m - SKEW, dTcs[0])
            # P(0) before k(1,0): chunk-1 activations are still in flight
            # when chunk 0 ends, so the P-block fills that DMA wait. For
            # later chunks the acts are long resident and P runs after
            # k(c,0) to hide the dT8 ACT lag instead.
            if P0_FIRST:
                p_block(0, dTcs[0])
                mb_mms(0)
            pend = None
            for c in range(1, NCH):
                for m in range(NM):
                    k_loop(c, m, dTcs[c % 2])
                    if m == 0 and (c > 1 or not P0_FIRST):
                        pend = p_head(c - 1, dTcs[(c - 1) % 2])
                    if m == 1 and pend is not None:
                        p_rest(c - 1, dTcs[(c - 1) % 2], pend)
                        mb_mms(c - 1)
                        pend = None
            p_block(NCH - 1, dTcs[(NCH - 1) % 2])
            mb_mms(NCH - 1)
            nc.scalar.copy(fm[:, 16:32], mbp[:, 0:16])
            _mp.close()
            nc.sync.dma_start(d_fm, fm[:])

    nc.compile()
    return nc


def _get_program(db_nonzero: bool):
    key = bool(db_nonzero)
    if key not in _PROGRAM_CACHE:
        _PROGRAM_CACHE[key] = _build_program(key)
    return _PROGRAM_CACHE[key]


# ----------------------------------------------------------------------------
# host side
# ----------------------------------------------------------------------------

def _host_scan_all(tg_all, sg_all, mask_f, gumbel):
    """Method-A sampling scan, all cores vectorized. Exact argmax semantics.
    Returns (wsel[B,S,E] f32, wsum f64, t_counts[E] f64, s_counts[E] f64)."""
    f32 = np.float32
    p = tg_all.astype(f32).copy()
    wsel = np.zeros((B, S, E), f32)
    s_counts = np.zeros(E, np.float64)
    BIG = f32(1e4)
    iota = np.arange(E, dtype=f32)
    for k in range(K):
        z = np.log(p) + gumbel[k]
        m = z.max(-1, keepdims=True)
        ge = (z >= m).astype(f32)
        t = iota + BIG - BIG * ge
        idxf = t.min(-1, keepdims=True)
        oh = (iota == idxf).astype(f32)
        po = p * oh
        w = po.sum(-1)
        sg_k = (sg_all * oh).sum(-1)
        mw = mask_f * w
        wsel += mw[..., None] * oh
        s_counts += ((mask_f * sg_k)[..., None] * oh).astype(np.float64).sum(axis=(0, 1))
        if k < K - 1:
            pn = p + (ALPHA - 1.0) * po
            p = pn / pn.sum(-1, keepdims=True)
    t_counts = wsel.astype(np.float64).sum(axis=(0, 1))
    wsum = float(t_counts.sum())
    return wsel, wsum, t_counts, s_counts


def _host_method_b(tg, sg, temp_c):
    """Per-core method-B partials: (tkl, ent)."""
    f32 = np.float32
    tg = tg.astype(f32)
    sg = sg.astype(f32)
    sgT = sg / f32(temp_c)
    ltg = np.log(tg)
    lsg = np.log(sg)
    ent = (sg * lsg).sum(dtype=f32)
    mb2 = sgT.max(-1, keepdims=True)
    ex = np.exp(sgT - mb2)
    se = ex.sum(-1, keepdims=True, dtype=f32)
    lse = np.log(se) + mb2
    sum_tg = tg.sum(-1, keepdims=True, dtype=f32)
    tkl = (tg * (ltg - sgT)).sum(dtype=f32) + (lse * sum_tg).sum(dtype=f32)
    return tkl, ent


def _host_quad(a_s_all, a_t_all, wsel, B_s, B_t):
    """Sum over tokens/experts of wsel * quad / H (the LoRA Gram term)."""
    G_ss = np.einsum("ehr,ehq->erq", B_s, B_s)
    G_st = np.einsum("ehr,ehq->erq", B_s, B_t)
    G_tt = np.einsum("ehr,ehq->erq", B_t, B_t)
    acc = 0.0
    for e in range(E):
        q1 = ((a_s_all @ G_ss[e]) * a_s_all).sum(-1)
        q2 = ((a_s_all @ G_st[e]) * a_t_all).sum(-1)
        q3 = ((a_t_all @ G_tt[e]) * a_t_all).sum(-1)
        qe = (SCALE_S * SCALE_S) * q1 - (2.0 * SCALE_S * SCALE_T) * q2 \
            + (SCALE_T * SCALE_T) * q3
        acc += float((wsel[:, :, e].astype(np.float64) * qe).sum())
    return acc / H


def _prep_shared(inputs, db_nonzero):
    """Replicated (per-core identical) device arrays."""
    f32 = np.float32
    W_t = np.asarray(inputs["W_t"], f32)
    W_s = np.asarray(inputs["W_s"], f32)
    B_t = np.asarray(inputs["B_t"], f32)
    B_s = np.asarray(inputs["B_s"], f32)
    db = (np.asarray(inputs["b_s"], f32) - np.asarray(inputs["b_t"], f32))

    nke = NK2 + (1 if db_nonzero else 0)

    # W layout [p, m, k2, j, c] = WSC * W[m*128+c, k2*256+j*128+p]
    def w_host(W, k_tiles, bias=None):
        out = np.zeros((128, NM, k_tiles, 2, 128), F8)
        out[:, :, :NK2] = (WSC * W).astype(F8).reshape(
            NM, 128, NK2, 2, 128).transpose(4, 0, 2, 3, 1)
        if bias is not None and k_tiles > NK2:
            out[0, :, NK2, 0, :] = (WSC * bias).astype(F8).reshape(NM, 128)
        return np.ascontiguousarray(out)

    Ws8 = w_host(W_s, nke, db if db_nonzero else None)
    Wt8 = w_host(-W_t, NK2)   # negated: PSUM accumulation adds, d = base_s - base_t

    # Bc8 [p, m2, j, col] = BSC * [Bs_her | Bt_her][m2*256+j*128+p, col]
    Bs_her = B_s.transpose(1, 0, 2).reshape(H, E * R)
    Bt_her = B_t.transpose(1, 0, 2).reshape(H, E * R)
    B_cat = np.concatenate([Bs_her, Bt_her], axis=1)
    Bc8 = np.ascontiguousarray(
        (BSC * B_cat).astype(F8).reshape(NMP, 2, 128, 256).transpose(2, 0, 1, 3))

    onesH = np.ones((128, 1), BF16)

    shared = dict(Ws8=Ws8, Wt8=Wt8, Bc8=Bc8, onesH=onesH)
    return shared, nke


def _prep_core(sh, th, a_s, a_t, nke, wsel, db_nonzero):
    """Per-core device arrays (plus host-side wsel_e for the mb combine)."""
    f32 = np.float32

    # wacat[t, (half, e, r)] = wsel[t, e] * f_half * a_half[t, r]: folds the
    # expert selection weights and cross-term scales into the V consume.
    fs = f32(2.0 * SCALE_S / (H * VSC))
    ft = f32(-2.0 * SCALE_T / (H * VSC))
    wa_s = wsel[:, :, None] * (fs * a_s)[:, None, :]       # [S, E, R]
    wa_t = wsel[:, :, None] * (ft * a_t)[:, None, :]
    wa = np.concatenate([wa_s.reshape(S, E * R), wa_t.reshape(S, E * R)],
                        axis=1)                            # [S, 256]
    wacat = np.ascontiguousarray(
        wa.reshape(NCHUNK, 128, 256).transpose(1, 0, 2)).astype(BF16)

    # [p, k2, j, s] layout of x.T
    def xt_host(x, k_tiles, ones_tail=False):
        out = np.zeros((128, k_tiles, 2, S), F8)
        out[:, :NK2] = x.T.astype(F8).reshape(NK2, 2, 128, S).transpose(2, 0, 1, 3)
        if ones_tail and k_tiles > NK2:
            out[0, NK2, 0, :] = F8(1.0)
        return np.ascontiguousarray(out)

    sh8 = xt_host(sh, nke, ones_tail=(nke > NK2))
    th8 = xt_host(th, NK2)

    wsel_e = np.ascontiguousarray(
        wsel.sum(-1).reshape(NCHUNK, 128).T).astype(f32)   # [128, NCHUNK]
    dev = dict(sh8=sh8, th8=th8, wacat=wacat)
    return dev, wsel_e


def _combine(feat_parts, feat_quad, wsum, t_counts, s_counts, tkls, ents, temp_c):
    f32 = np.float32
    feat = np.sum(np.asarray(feat_parts, f32), dtype=f32) + f32(feat_quad)
    tc = np.asarray(t_counts, np.float64)
    sc = np.asarray(s_counts, np.float64)
    tkl = np.sum(np.asarray(tkls, f32), dtype=f32)
    ent = np.sum(np.asarray(ents, f32), dtype=f32)

    feat_loss = feat / max(wsum, 1e-8)
    t_avg = tc / tc.sum() + EPS
    s_avg = sc / sc.sum() + EPS
    t_avg = t_avg / t_avg.sum()
    s_avg = s_avg / s_avg.sum()
    coverage_kl = (t_avg * (np.log(t_avg) - np.log(s_avg))).sum() / E
    method_a_total = feat_loss + LAMBDA_COV * coverage_kl
    temp_kl = tkl / B
    entropy_loss = ent / (B * S)
    method_b_total = temp_kl + BETA_ENT * entropy_loss
    return np.array(
        [feat_loss, coverage_kl, method_a_total, temp_kl, entropy_loss,
         method_b_total, temp_c], f32)


def _host_all(inputs):
    """Host scan/method-B/quad for all cores + per-core device input maps."""
    f32 = np.float32
    db_nonzero = bool(
        np.any(np.asarray(inputs["b_s"], f32) != np.asarray(inputs["b_t"], f32)))
    temp = float(np.asarray(inputs["temperature"], f32))
    temp_c = float(np.clip(temp, TEMP_LO, TEMP_HI))

    u = np.asarray(inputs["uniform_noise"], f32)
    gumbel = -np.log(-np.log(u * (1.0 - 2e-7) + 1e-7)).astype(f32)
    mask_f = np.asarray(inputs["attention_mask"], f32)
    tg_all = np.asarray(inputs["teacher_gates"], f32)
    sg_all = np.asarray(inputs["student_gates"], f32)

    shared, nke = _prep_shared(inputs, db_nonzero)
    wsel_all, wsum, t_counts, s_counts = _host_scan_all(
        tg_all, sg_all, mask_f, gumbel)

    A_sT = np.ascontiguousarray(np.asarray(inputs["A_s"], f32).T)
    A_tT = np.ascontiguousarray(np.asarray(inputs["A_t"], f32).T)
    sh_all = np.asarray(inputs["student_hidden_states"], f32)
    th_all = np.asarray(inputs["teacher_hidden_states"], f32)

    in_maps = []
    tkls, ents, wsel_es = [], [], []
    a_s_all = np.empty((B, S, R), f32)
    a_t_all = np.empty((B, S, R), f32)
    for c in range(B):
        tkl, ent = _host_method_b(tg_all[c], sg_all[c], temp_c)
        tkls.append(tkl)
        ents.append(ent)
        a_s = sh_all[c] @ A_sT
        a_t = th_all[c] @ A_tT
        a_s_all[c] = a_s
        a_t_all[c] = a_t
        m = dict(shared)
        dev, wsel_e = _prep_core(sh_all[c], th_all[c], a_s, a_t, nke,
                                 wsel_all[c], db_nonzero)
        m.update(dev)
        in_maps.append(m)
        wsel_es.append(wsel_e)

    feat_quad = _host_quad(a_s_all, a_t_all, wsel_all,
                           np.asarray(inputs["B_s"], f32),
                           np.asarray(inputs["B_t"], f32))

    return dict(in_maps=in_maps, db_nonzero=db_nonzero, temp_c=temp_c,
                wsum=wsum, t_counts=t_counts, s_counts=s_counts,
                tkls=tkls, ents=ents, feat_quad=feat_quad, wsel_es=wsel_es)


def kernel(**inputs) -> np.ndarray:
    host = _host_all(inputs)
    nc = _get_program(host["db_nonzero"])

    from concourse.bass_utils import run_bass_kernel_spmd

    res = run_bass_kernel_spmd(nc, host["in_maps"], core_ids=list(range(B)))
    feat_parts = []
    for c in range(B):
        fm = np.asarray(res.results[c]["fm"], np.float32)   # [128, 32]
        fmse = fm[:, 0:16]
        mb = fm[:, 16:32]
        feat_parts.append(float(fmse.sum(dtype=np.float64))
                          + float((mb * host["wsel_es"][c]).sum(dtype=np.float64))
                          / (H * WSC * WSC))

    return _combine(feat_parts, host["feat_quad"], host["wsum"],
                    host["t_counts"], host["s_counts"], host["tkls"],
                    host["ents"], host["temp_c"])


# revision 5
# speedup vs baseline: 3.7723x; 1.0031x over previous
"""Trainium2 Bass kernel for nn_ExpertDistillationLoss — fp8 DoubleRow version.

Strategy (data-parallel over batch, 8 cores, 1 batch element each):
  - Device (per core): the FLOP-heavy expert-MSE pipeline in fp8e4 DoubleRow
    (2 fp8 weights/cell, 256-deep contraction per matmul, 0.5 cycles/row):
      pd[c, t] = 64*(sh@Ws.T - th@Wt.T) via 16 DR matmuls per (m, chunk),
      weights fully resident in SBUF (fp8, 64 KB/partition total).
      mean_base: ACT square of pd -> DVE accumulate -> per-chunk ones-matmuls.
      cross: ACT downcast dT8 = pd/4 (fp8) -> DR P-matmuls against resident
      Bc8 (m-tile pairs) into V[t, 256] -> DVE multiply/reduce vs host-scaled
      acat. Device output per core: feat partial (mean_base+cross) scalar.
  - Host: input sharding + fp8 layout prep, the K=3 MC sampling scan (exact
    argmax semantics), the quad (LoRA Gram) term at sampled experts,
    method-B losses, and the final scalar combine.
"""

import numpy as np
import ml_dtypes

B, S, H, E, R, K = 8, 2048, 2048, 8, 16, 3
ALPHA = 0.5
LAMBDA_COV = 0.5
BETA_ENT = 0.1
TEMP_LO, TEMP_HI = 0.5, 1.5
SCALE_T = 2.0
SCALE_S = 2.0
EPS = 1e-8

NK2 = H // 256         # 8 k2-tiles (256-deep DoubleRow contraction)
NM = H // 128          # 16 output h-tiles
NMP = NM // 2          # 8 m-tile pairs for the P-matmul
NC = S // 512          # 4 s-chunks of 512
NSUB = 4               # 128-token subchunks per s-chunk
NCHUNK = S // 128      # 16

WSC = 64.0             # weight prescale (power of 2; keeps fp8 W out of subnormals)
DSC = 0.25             # dT8 = DSC * pd  (pd = WSC*d, so dT8 = 16*d)
BSC = 256.0            # B prescale for fp8
VSC = (WSC * DSC) * BSC    # 4096: scale carried by V = dT8^T Bc8

import os
N_WARM = int(os.environ.get("KV2_WARM", "40"))   # PE clock-warmup dummies
N_WARM2 = int(os.environ.get("KV2_WARM2", "0"))  # second burst (th chunk-0)
P0_FIRST = os.environ.get("KV2_P0FIRST", "0") == "1"

BF16 = ml_dtypes.bfloat16
F8 = ml_dtypes.float8_e4m3

_PROGRAM_CACHE = {}


# ----------------------------------------------------------------------------
# device program
# ----------------------------------------------------------------------------

def _build_program(db_nonzero: bool):
    import os
    import concourse.bacc as bacc
    import concourse.tile as tile
    from concourse import mybir

    f32 = mybir.dt.float32
    fp8 = mybir.dt.float8e4
    ALU = mybir.AluOpType
    AX = mybir.AxisListType
    DR = mybir.MatmulPerfMode.DoubleRow

    nke = NK2 + (1 if db_nonzero else 0)   # extra k2-tile carries the bias row

    nc = bacc.Bacc("TRN2", target_bir_lowering=False, debug=False)

    # DRAM inputs (per-core shapes; layouts are host-prepared)
    d_sh = nc.dram_tensor("sh8", [128, nke, 2, S], fp8, kind="ExternalInput").ap()
    d_th = nc.dram_tensor("th8", [128, NK2, 2, S], fp8, kind="ExternalInput").ap()
    d_Ws = nc.dram_tensor("Ws8", [128, NM, nke, 2, 128], fp8, kind="ExternalInput").ap()
    d_Wt = nc.dram_tensor("Wt8", [128, NM, NK2, 2, 128], fp8, kind="ExternalInput").ap()
    d_Bc = nc.dram_tensor("Bc8", [128, NMP, 2, 256], fp8, kind="ExternalInput").ap()
    bf16 = mybir.dt.bfloat16
    d_wa = nc.dram_tensor("wacat", [128, NCHUNK, 256], bf16, kind="ExternalInput").ap()
    d_onesH = nc.dram_tensor("onesH", [128, 1], bf16, kind="ExternalInput").ap()

    d_fm = nc.dram_tensor("fm", [128, 32], f32, kind="ExternalOutput").ap()

    with tile.TileContext(nc) as tc:
        with (
            tc.tile_pool(name="const", bufs=1) as cp,
            tc.tile_pool(name="sq", bufs=2) as qp,
            tc.tile_pool(name="vc", bufs=2) as vp,
        ):
            from contextlib import ExitStack
            _mp = ExitStack()
            pd = _mp.enter_context(tc.tile_pool(name="pd", bufs=int(__import__("os").environ.get("KV2_PDB","4")), space="PSUM"))
            pv = _mp.enter_context(tc.tile_pool(name="pv", bufs=int(__import__("os").environ.get("KV2_PVB","3")), space="PSUM"))
            pm = _mp.enter_context(tc.tile_pool(name="pm", bufs=1, space="PSUM"))

            # ---- PE clock warmup: dummy DR matmuls on memset tiles run
            # while the first real DMAs are in flight, so the p-state ramp
            # completes before the first real k-loop.
            dwu = cp.tile([128, 256], fp8, tag="dwu")
            nc.vector.memset(dwu[:], 0.0)
            dpd = pd.tile([128, 128], f32, tag="pd", name="warm_pd")
            dwv = dwu[:].rearrange("p (j c) -> p j c", j=2)
            if N_WARM:
                for i in range(N_WARM):
                    nc.tensor.matmul(dpd[:], dwv, dwv,
                                     start=(i == 0), stop=(i == N_WARM - 1),
                                     perf_mode=DR)

            def warm2():
                if not N_WARM2:
                    return
                # pv pool: its banks are untouched until the first P-block,
                # so this group cannot collide with the open pd k-loop groups
                dpd2 = pv.tile([128, 128], f32, tag="V", name="warm_pd2")
                for i in range(N_WARM2):
                    nc.tensor.matmul(dpd2[:], dwv, dwv,
                                     start=(i == 0), stop=(i == N_WARM2 - 1),
                                     perf_mode=DR)

            # ---- resident loads; emission order = DMA service order ----
            Ws = cp.tile([128, NM * nke * 2 * 128], fp8, tag="Ws")
            Wt = cp.tile([128, NM * NK2 * 2 * 128], fp8, tag="Wt")
            sh = cp.tile([128, nke * 2 * S], fp8, tag="sh")
            th = cp.tile([128, NK2 * 2 * S], fp8, tag="th")
            wstr = nke * 2 * 128       # Ws stride per m
            wttr = NK2 * 2 * 128       # Wt stride per m
            d_Wsf = d_Ws[:].rearrange("p m k j c -> p (m k j c)")
            d_Wtf = d_Wt[:].rearrange("p m k j c -> p (m k j c)")
            shv = sh[:].rearrange("p (k j s) -> p k j s", k=nke, j=2)
            thv = th[:].rearrange("p (k j s) -> p k j s", k=NK2, j=2)

            def load_w(m):
                nc.sync.dma_start(Ws[:, m * wstr:(m + 1) * wstr],
                                  d_Wsf[:, m * wstr:(m + 1) * wstr])
                nc.sync.dma_start(Wt[:, m * wttr:(m + 1) * wttr],
                                  d_Wtf[:, m * wttr:(m + 1) * wttr])

            def load_act(c):
                s0, s1 = c * 512, (c + 1) * 512
                nc.sync.dma_start(shv[:, :, :, s0:s1], d_sh[:, :, :, s0:s1])
                nc.sync.dma_start(thv[:, :, :, s0:s1], d_th[:, :, :, s0:s1])

            head_eng = nc.gpsimd if os.environ.get("KV2_HEADDMA", "sync") == "gps" \
                else nc.sync
            head_eng.dma_start(Ws[:, 0:wstr], d_Wsf[:, 0:wstr])
            head_eng.dma_start(shv[:, :, :, 0:512], d_sh[:, :, :, 0:512])
            head_eng.dma_start(Wt[:, 0:wttr], d_Wtf[:, 0:wttr])
            head_eng.dma_start(thv[:, :, :, 0:512], d_th[:, :, :, 0:512])
            wa_sb = cp.tile([128, NCHUNK * 256], bf16, tag="wa")

            def load_wa(c):
                nc.sync.dma_start(
                    wa_sb[:, c * 1024:(c + 1) * 1024],
                    d_wa[:].rearrange("p a b -> p (a b)")[:, c * 1024:(c + 1) * 1024])

            # straight weight stream: supply (1.46us/m-pair) stays ahead of
            # the steady k-loop demand (1.71us/m), so no mid-chunk stalls;
            # Bc and the chunk-1 acts land just before P(0)/k(1,0) need them
            for m in range(1, NM):
                load_w(m)
            Bc = cp.tile([128, NMP * 2 * 256], fp8, tag="Bc")
            nc.sync.dma_start(Bc[:], d_Bc[:].rearrange("p a b c -> p (a b c)"))
            load_act(1)
            load_act(2)
            load_act(3)
            for c in range(NC):
                load_wa(c)
            onesH = cp.tile([128, 1], bf16, tag="onesH")
            nc.sync.dma_start(onesH[:], d_onesH)

            acc128 = cp.tile([128, S], bf16, tag="acc128")
            nc.vector.memset(acc128[:], 0.0)
            fm = cp.tile([128, 32], f32, tag="fm")   # [fmse(16) | mb(16)]
            mbp = pm.tile([128, 16], f32, tag="pmisc")

            dTc0 = cp.tile([128, NM * 512], fp8, tag="dTc0")
            dTc1 = cp.tile([128, NM * 512], fp8, tag="dTc1")

            Wsv = Ws[:].rearrange("p (m k j c) -> p m k j c", m=NM, k=nke, j=2)
            Wtv = Wt[:].rearrange("p (m k j c) -> p m k j c", m=NM, k=NK2, j=2)

            CHUNKS = [(c * 512, 512) for c in range(NC)]
            NCH = len(CHUNKS)

            pds = {}

            def k_s(c, m):
                s0, w = CHUNKS[c]
                pd_t = pd.tile([128, w], f32, tag="pd", name=f"pd_{c}_{m}")
                pds[(c, m)] = pd_t
                for k2 in range(nke):
                    nc.tensor.matmul(pd_t[:], Wsv[:, m, k2],
                                     shv[:, k2, :, s0:s0 + w],
                                     start=(k2 == 0), stop=False, perf_mode=DR)

            def k_t(c, m, dTc):
                s0, w = CHUNKS[c]
                pd_t = pds.pop((c, m))
                for k2 in range(NK2):
                    nc.tensor.matmul(pd_t[:], Wtv[:, m, k2],
                                     thv[:, k2, :, s0:s0 + w],
                                     start=False, stop=(k2 == NK2 - 1),
                                     perf_mode=DR)
                nc.scalar.mul(dTc[:, m * 512:m * 512 + w], pd_t[:], DSC)
                sq = qp.tile([128, w], bf16, tag="sq", name=f"sq_{c}_{m}")
                nc.scalar.square(sq[:], pd_t[:])
                # bf16 accumulate: DVE 2x mode (all-16-bit packed SBUF ops);
                # ~0.3% noise on mean_base, far inside the 2e-2 budget
                with nc.allow_low_precision("bf16 mean-base accumulate"):
                    nc.vector.tensor_add(acc128[:, s0:s0 + w],
                                         acc128[:, s0:s0 + w], sq[:])

            def k_loop(c, m, dTc):
                k_s(c, m)
                k_t(c, m, dTc)

            ACT_RED_SUBS = tuple(int(x) for x in os.environ.get(
                "KV2_ACTRED", "0,1,2").split(",") if x != "")

            def consume_v(Vt, chunk, sub):
                # wacat already carries wsel * a * scale: one multiply + one
                # full-width reduce per 128-token chunk. Multiplies alternate
                # DVE / GpSimd and reduces DVE / ACT-accumulator, spreading
                # the final chunk's drain across three engines.
                prod = vp.tile([128, 256], f32, tag="prod",
                               name=f"prod_{chunk}")
                nc.vector.tensor_tensor(prod[:], Vt[:],
                                        wa_sb[:, chunk * 256:(chunk + 1) * 256],
                                        ALU.mult)
                if sub in ACT_RED_SUBS:
                    # ACT accumulator takes one reduce off DVE; the LAST
                    # sub's reduce stays on DVE so the tail's final hop
                    # avoids a cross-engine semaphore
                    nc.scalar.activation(
                        prod[:], prod[:], mybir.ActivationFunctionType.Copy,
                        accum_out=fm[:, chunk:chunk + 1])
                else:
                    nc.vector.tensor_reduce(fm[:, chunk:chunk + 1], prod[:],
                                            axis=AX.X, op=ALU.add)

            # P-block split: the head (m-pairs 0..5) depends only on dT8
            # tiles the ACT queue finished long ago; the rest (pairs 6, 7 +
            # consume) waits on the final dT8 copies and is emitted one
            # k-loop later so PE never idles on the ACT lag.
            def p_head(c, dTc):
                nsub = CHUNKS[c][1] // 128
                dv = dTc[:].rearrange("p (m s) -> p m s", m=NM)
                Bv = Bc[:].rearrange("p (a b c) -> p a b c", a=NMP, b=2)
                Vts = []
                for sub in range(nsub):
                    t0 = sub * 128
                    Vt = pv.tile([128, 256], f32, tag="V", name=f"V_{c}_{sub}")
                    Vts.append(Vt)
                    for m2 in range(NMP - 2):
                        nc.tensor.matmul(
                            Vt[:],
                            dv[:, 2 * m2:2 * m2 + 2, t0:t0 + 128],
                            Bv[:, m2],
                            start=(m2 == 0), stop=False,
                            perf_mode=DR)
                return Vts

            def p_rest(c, dTc, Vts):
                s0, w = CHUNKS[c]
                dv = dTc[:].rearrange("p (m s) -> p m s", m=NM)
                Bv = Bc[:].rearrange("p (a b c) -> p a b c", a=NMP, b=2)
                for sub in range(w // 128):
                    t0 = sub * 128
                    for m2 in (NMP - 2, NMP - 1):
                        nc.tensor.matmul(
                            Vts[sub][:],
                            dv[:, 2 * m2:2 * m2 + 2, t0:t0 + 128],
                            Bv[:, m2],
                            start=False, stop=(m2 == NMP - 1),
                            perf_mode=DR)
                    consume_v(Vts[sub], s0 // 128 + sub, sub)

            def p_block(c, dTc):
                p_rest(c, dTc, p_head(c, dTc))

            def mb_mms(c):
                # per-128-token-chunk ones-matmuls over this chunk's acc128
                s0, w = CHUNKS[c]
                for i in range(w // 128):
                    cc = s0 // 128 + i
                    nc.tensor.matmul(mbp[:, cc:cc + 1],
                                     acc128[:, cc * 128:(cc + 1) * 128],
                                     onesH[:], start=True, stop=True)

            # ---- software-pipelined main loop ----
            # chunk 0 runs the s-matmuls 3 m-tiles ahead of the t-matmuls so
            # PE need not wait for the th chunk-0 DMA; later chunks pair s/t.
            # P-matmuls of chunk c are emitted after k(c+1, m=0) so they never
            # wait on the in-flight ACT dT8 copies.
            dTcs = [dTc0, dTc1]
            SKEW = int(os.environ.get("KV2_SKEW", "1"))
            for m in range(NM + SKEW):
                if m < NM:
                    k_s(0, m)
                if m == SKEW - 1:
                    warm2()
                if m >= SKEW:
                    k_t(0, m - SKEW, dTcs[0])
            # P(0) before k(1,0): chunk-1 activations are still in flight
            # when chunk 0 ends, so the P-block fills that DMA wait. For
            # later chunks the acts are long resident and P runs after
            # k(c,0) to hide the dT8 ACT lag instead.
            if P0_FIRST:
                p_block(0, dTcs[0])
                mb_mms(0)
            pend = None
            for c in range(1, NCH):
                for m in range(NM):
                    k_loop(c, m, dTcs[c % 2])
                    if m == 0 and (c > 1 or not P0_FIRST):
                        pend = p_head(c - 1, dTcs[(c - 1) % 2])
                    if m == 1 and pend is not None:
                        p_rest(c - 1, dTcs[(c - 1) % 2], pend)
                        mb_mms(c - 1)
                        pend = None
            p_block(NCH - 1, dTcs[(NCH - 1) % 2])
            mb_mms(NCH - 1)
            nc.scalar.copy(fm[:, 16:32], mbp[:, 0:16])
            _mp.close()
            nc.sync.dma_start(d_fm, fm[:])

    nc.compile()
    return nc


def _get_program(db_nonzero: bool):
    key = bool(db_nonzero)
    if key not in _PROGRAM_CACHE:
        _PROGRAM_CACHE[key] = _build_program(key)
    return _PROGRAM_CACHE[key]


# ----------------------------------------------------------------------------
# host side
# ----------------------------------------------------------------------------

def _host_scan_all(tg_all, sg_all, mask_f, gumbel):
    """Method-A sampling scan, all cores vectorized. Exact argmax semantics.
    Returns (wsel[B,S,E] f32, wsum f64, t_counts[E] f64, s_counts[E] f64)."""
    f32 = np.float32
    p = tg_all.astype(f32).copy()
    wsel = np.zeros((B, S, E), f32)
    s_counts = np.zeros(E, np.float64)
    BIG = f32(1e4)
    iota = np.arange(E, dtype=f32)
    for k in range(K):
        z = np.log(p) + gumbel[k]
        m = z.max(-1, keepdims=True)
        ge = (z >= m).astype(f32)
        t = iota + BIG - BIG * ge
        idxf = t.min(-1, keepdims=True)
        oh = (iota == idxf).astype(f32)
        po = p * oh
        w = po.sum(-1)
        sg_k = (sg_all * oh).sum(-1)
        mw = mask_f * w
        wsel += mw[..., None] * oh
        s_counts += ((mask_f * sg_k)[..., None] * oh).astype(np.float64).sum(axis=(0, 1))
        if k < K - 1:
            pn = p + (ALPHA - 1.0) * po
            p = pn / pn.sum(-1, keepdims=True)
    t_counts = wsel.astype(np.float64).sum(axis=(0, 1))
    wsum = float(t_counts.sum())
    return wsel, wsum, t_counts, s_counts


def _host_method_b(tg, sg, temp_c):
    """Per-core method-B partials: (tkl, ent)."""
    f32 = np.float32
    tg = tg.astype(f32)
    sg = sg.astype(f32)
    sgT = sg / f32(temp_c)
    ltg = np.log(tg)
    lsg = np.log(sg)
    ent = (sg * lsg).sum(dtype=f32)
    mb2 = sgT.max(-1, keepdims=True)
    ex = np.exp(sgT - mb2)
    se = ex.sum(-1, keepdims=True, dtype=f32)
    lse = np.log(se) + mb2
    sum_tg = tg.sum(-1, keepdims=True, dtype=f32)
    tkl = (tg * (ltg - sgT)).sum(dtype=f32) + (lse * sum_tg).sum(dtype=f32)
    return tkl, ent


def _host_quad(a_s_all, a_t_all, wsel, B_s, B_t):
    """Sum over tokens/experts of wsel * quad / H (the LoRA Gram term)."""
    G_ss = np.einsum("ehr,ehq->erq", B_s, B_s)
    G_st = np.einsum("ehr,ehq->erq", B_s, B_t)
    G_tt = np.einsum("ehr,ehq->erq", B_t, B_t)
    acc = 0.0
    for e in range(E):
        q1 = ((a_s_all @ G_ss[e]) * a_s_all).sum(-1)
        q2 = ((a_s_all @ G_st[e]) * a_t_all).sum(-1)
        q3 = ((a_t_all @ G_tt[e]) * a_t_all).sum(-1)
        qe = (SCALE_S * SCALE_S) * q1 - (2.0 * SCALE_S * SCALE_T) * q2 \
            + (SCALE_T * SCALE_T) * q3
        acc += float((wsel[:, :, e].astype(np.float64) * qe).sum())
    return acc / H


def _prep_shared(inputs, db_nonzero):
    """Replicated (per-core identical) device arrays."""
    f32 = np.float32
    W_t = np.asarray(inputs["W_t"], f32)
    W_s = np.asarray(inputs["W_s"], f32)
    B_t = np.asarray(inputs["B_t"], f32)
    B_s = np.asarray(inputs["B_s"], f32)
    db = (np.asarray(inputs["b_s"], f32) - np.asarray(inputs["b_t"], f32))

    nke = NK2 + (1 if db_nonzero else 0)

    # W layout [p, m, k2, j, c] = WSC * W[m*128+c, k2*256+j*128+p]
    def w_host(W, k_tiles, bias=None):
        out = np.zeros((128, NM, k_tiles, 2, 128), F8)
        out[:, :, :NK2] = (WSC * W).astype(F8).reshape(
            NM, 128, NK2, 2, 128).transpose(4, 0, 2, 3, 1)
        if bias is not None and k_tiles > NK2:
            out[0, :, NK2, 0, :] = (WSC * bias).astype(F8).reshape(NM, 128)
        return np.ascontiguousarray(out)

    Ws8 = w_host(W_s, nke, db if db_nonzero else None)
    Wt8 = w_host(-W_t, NK2)   # negated: PSUM accumulation adds, d = base_s - base_t

    # Bc8 [p, m2, j, col] = BSC * [Bs_her | Bt_her][m2*256+j*128+p, col]
    Bs_her = B_s.transpose(1, 0, 2).reshape(H, E * R)
    Bt_her = B_t.transpose(1, 0, 2).reshape(H, E * R)
    B_cat = np.concatenate([Bs_her, Bt_her], axis=1)
    Bc8 = np.ascontiguousarray(
        (BSC * B_cat).astype(F8).reshape(NMP, 2, 128, 256).transpose(2, 0, 1, 3))

    onesH = np.ones((128, 1), BF16)

    shared = dict(Ws8=Ws8, Wt8=Wt8, Bc8=Bc8, onesH=onesH)
    return shared, nke


def _prep_core(sh, th, a_s, a_t, nke, wsel, db_nonzero):
    """Per-core device arrays (plus host-side wsel_e for the mb combine)."""
    f32 = np.float32

    # wacat[t, (half, e, r)] = wsel[t, e] * f_half * a_half[t, r]: folds the
    # expert selection weights and cross-term scales into the V consume.
    fs = f32(2.0 * SCALE_S / (H * VSC))
    ft = f32(-2.0 * SCALE_T / (H * VSC))
    wa_s = wsel[:, :, None] * (fs * a_s)[:, None, :]       # [S, E, R]
    wa_t = wsel[:, :, None] * (ft * a_t)[:, None, :]
    wa = np.concatenate([wa_s.reshape(S, E * R), wa_t.reshape(S, E * R)],
                        axis=1)                            # [S, 256]
    wacat = np.ascontiguousarray(
        wa.reshape(NCHUNK, 128, 256).transpose(1, 0, 2)).astype(BF16)

    # [p, k2, j, s] layout of x.T
    def xt_host(x, k_tiles, ones_tail=False):
        out = np.zeros((128, k_tiles, 2, S), F8)
        out[:, :NK2] = x.T.astype(F8).reshape(NK2, 2, 128, S).transpose(2, 0, 1, 3)
        if ones_tail and k_tiles > NK2:
            out[0, NK2, 0, :] = F8(1.0)
        return np.ascontiguousarray(out)

    sh8 = xt_host(sh, nke, ones_tail=(nke > NK2))
    th8 = xt_host(th, NK2)

    wsel_e = np.ascontiguousarray(
        wsel.sum(-1).reshape(NCHUNK, 128).T).astype(f32)   # [128, NCHUNK]
    dev = dict(sh8=sh8, th8=th8, wacat=wacat)
    return dev, wsel_e


def _combine(feat_parts, feat_quad, wsum, t_counts, s_counts, tkls, ents, temp_c):
    f32 = np.float32
    feat = np.sum(np.asarray(feat_parts, f32), dtype=f32) + f32(feat_quad)
    tc = np.asarray(t_counts, np.float64)
    sc = np.asarray(s_counts, np.float64)
    tkl = np.sum(np.asarray(tkls, f32), dtype=f32)
    ent = np.sum(np.asarray(ents, f32), dtype=f32)

    feat_loss = feat / max(wsum, 1e-8)
    t_avg = tc / tc.sum() + EPS
    s_avg = sc / sc.sum() + EPS
    t_avg = t_avg / t_avg.sum()
    s_avg = s_avg / s_avg.sum()
    coverage_kl = (t_avg * (np.log(t_avg) - np.log(s_avg))).sum() / E
    method_a_total = feat_loss + LAMBDA_COV * coverage_kl
    temp_kl = tkl / B
    entropy_loss = ent / (B * S)
    method_b_total = temp_kl + BETA_ENT * entropy_loss
    return np.array(
        [feat_loss, coverage_kl, method_a_total, temp_kl, entropy_loss,
         method_b_total, temp_c], f32)


def _host_all(inputs):
    """Host scan/method-B/quad for all cores + per-core device input maps."""
    f32 = np.float32
    db_nonzero = bool(
        np.any(np.asarray(inputs["b_s"], f32) != np.asarray(inputs["b_t"], f32)))
    temp = float(np.asarray(inputs["temperature"], f32))
    temp_c = float(np.clip(temp, TEMP_LO, TEMP_HI))

    u = np.asarray(inputs["uniform_noise"], f32)
    gumbel = -np.log(-np.log(u * (1.0 - 2e-7) + 1e-7)).astype(f32)
    mask_f = np.asarray(inputs["attention_mask"], f32)
    tg_all = np.asarray(inputs["teacher_gates"], f32)
    sg_all = np.asarray(inputs["student_gates"], f32)

    shared, nke = _prep_shared(inputs, db_nonzero)
    wsel_all, wsum, t_counts, s_counts = _host_scan_all(
        tg_all, sg_all, mask_f, gumbel)

    A_sT = np.ascontiguousarray(np.asarray(inputs["A_s"], f32).T)
    A_tT = np.ascontiguousarray(np.asarray(inputs["A_t"], f32).T)
    sh_all = np.asarray(inputs["student_hidden_states"], f32)
    th_all = np.asarray(inputs["teacher_hidden_states"], f32)

    in_maps = []
    tkls, ents, wsel_es = [], [], []
    a_s_all = np.empty((B, S, R), f32)
    a_t_all = np.empty((B, S, R), f32)
    for c in range(B):
        tkl, ent = _host_method_b(tg_all[c], sg_all[c], temp_c)
        tkls.append(tkl)
        ents.append(ent)
        a_s = sh_all[c] @ A_sT
        a_t = th_all[c] @ A_tT
        a_s_all[c] = a_s
        a_t_all[c] = a_t
        m = dict(shared)
        dev, wsel_e = _prep_core(sh_all[c], th_all[c], a_s, a_t, nke,
                                 wsel_all[c], db_nonzero)
        m.update(dev)
        in_maps.append(m)
        wsel_es.append(wsel_e)

    feat_quad = _host_quad(a_s_all, a_t_all, wsel_all,
                           np.asarray(inputs["B_s"], f32),
                           np.asarray(inputs["B_t"], f32))

    return dict(in_maps=in_maps, db_nonzero=db_nonzero, temp_c=temp_c,
                wsum=wsum, t_counts=t_counts, s_counts=s_counts,
                tkls=tkls, ents=ents, feat_quad=feat_quad, wsel_es=wsel_es)


def kernel(**inputs) -> np.ndarray:
    host = _host_all(inputs)
    nc = _get_program(host["db_nonzero"])

    from concourse.bass_utils import run_bass_kernel_spmd

    res = run_bass_kernel_spmd(nc, host["in_maps"], core_ids=list(range(B)))
    feat_parts = []
    for c in range(B):
        fm = np.asarray(res.results[c]["fm"], np.float32)   # [128, 32]
        fmse = fm[:, 0:16]
        mb = fm[:, 16:32]
        feat_parts.append(float(fmse.sum(dtype=np.float64))
                          + float((mb * host["wsel_es"][c]).sum(dtype=np.float64))
                          / (H * WSC * WSC))

    return _combine(feat_parts, host["feat_quad"], host["wsum"],
                    host["t_counts"], host["s_counts"], host["tkls"],
                    host["ents"], host["temp_c"])


# revision 7
# speedup vs baseline: 3.7981x; 1.0068x over previous
"""Trainium2 Bass kernel for nn_ExpertDistillationLoss — fp8 DoubleRow version.

Strategy (data-parallel over batch, 8 cores, 1 batch element each):
  - Device (per core): the FLOP-heavy expert-MSE pipeline in fp8e4 DoubleRow
    (2 fp8 weights/cell, 256-deep contraction per matmul, 0.5 cycles/row):
      pd[c, t] = 64*(sh@Ws.T - th@Wt.T) via 16 DR matmuls per (m, chunk),
      weights fully resident in SBUF (fp8, 64 KB/partition total).
      mean_base: ACT square of pd -> DVE accumulate -> per-chunk ones-matmuls.
      cross: ACT downcast dT8 = pd/4 (fp8) -> DR P-matmuls against resident
      Bc8 (m-tile pairs) into V[t, 256] -> DVE multiply/reduce vs host-scaled
      acat. Device output per core: feat partial (mean_base+cross) scalar.
  - Host: input sharding + fp8 layout prep, the K=3 MC sampling scan (exact
    argmax semantics), the quad (LoRA Gram) term at sampled experts,
    method-B losses, and the final scalar combine.
"""

import numpy as np
import ml_dtypes

B, S, H, E, R, K = 8, 2048, 2048, 8, 16, 3
ALPHA = 0.5
LAMBDA_COV = 0.5
BETA_ENT = 0.1
TEMP_LO, TEMP_HI = 0.5, 1.5
SCALE_T = 2.0
SCALE_S = 2.0
EPS = 1e-8

NK2 = H // 256         # 8 k2-tiles (256-deep DoubleRow contraction)
NM = H // 128          # 16 output h-tiles
NMP = NM // 2          # 8 m-tile pairs for the P-matmul
NC = S // 512          # 4 s-chunks of 512
NSUB = 4               # 128-token subchunks per s-chunk
NCHUNK = S // 128      # 16

WSC = 64.0             # weight prescale (power of 2; keeps fp8 W out of subnormals)
DSC = 0.25             # dT8 = DSC * pd  (pd = WSC*d, so dT8 = 16*d)
BSC = 256.0            # B prescale for fp8
VSC = (WSC * DSC) * BSC    # 4096: scale carried by V = dT8^T Bc8

import os
N_WARM = int(os.environ.get("KV2_WARM", "40"))   # PE clock-warmup dummies
N_WARM2 = int(os.environ.get("KV2_WARM2", "0"))  # second burst (th chunk-0)
P0_FIRST = os.environ.get("KV2_P0FIRST", "0") == "1"

BF16 = ml_dtypes.bfloat16
F8 = ml_dtypes.float8_e4m3

_PROGRAM_CACHE = {}


# ----------------------------------------------------------------------------
# device program
# ----------------------------------------------------------------------------

def _build_program(db_nonzero: bool):
    import os
    import concourse.bacc as bacc
    import concourse.tile as tile
    from concourse import mybir

    f32 = mybir.dt.float32
    fp8 = mybir.dt.float8e4
    ALU = mybir.AluOpType
    AX = mybir.AxisListType
    DR = mybir.MatmulPerfMode.DoubleRow

    nke = NK2 + (1 if db_nonzero else 0)   # extra k2-tile carries the bias row

    nc = bacc.Bacc("TRN2", target_bir_lowering=False, debug=False)

    # DRAM inputs (per-core shapes; layouts are host-prepared)
    d_sh = nc.dram_tensor("sh8", [128, nke, 2, S], fp8, kind="ExternalInput").ap()
    d_th = nc.dram_tensor("th8", [128, NK2, 2, S], fp8, kind="ExternalInput").ap()
    d_Ws = nc.dram_tensor("Ws8", [128, NM, nke, 2, 128], fp8, kind="ExternalInput").ap()
    d_Wt = nc.dram_tensor("Wt8", [128, NM, NK2, 2, 128], fp8, kind="ExternalInput").ap()
    d_Bc = nc.dram_tensor("Bc8", [128, NMP, 2, 256], fp8, kind="ExternalInput").ap()
    bf16 = mybir.dt.bfloat16
    d_wa = nc.dram_tensor("wacat", [128, NCHUNK, 256], bf16, kind="ExternalInput").ap()
    d_onesH = nc.dram_tensor("onesH", [128, 1], bf16, kind="ExternalInput").ap()

    d_fm = nc.dram_tensor("fm", [128, 32], f32, kind="ExternalOutput").ap()

    with tile.TileContext(nc) as tc:
        with (
            tc.tile_pool(name="const", bufs=1) as cp,
            tc.tile_pool(name="sq", bufs=int(os.environ.get("KV2_QPB", "2"))) as qp,
            tc.tile_pool(name="vc", bufs=int(os.environ.get("KV2_VCB", "3"))) as vp,
        ):
            from contextlib import ExitStack
            _mp = ExitStack()
            pd = _mp.enter_context(tc.tile_pool(name="pd", bufs=int(__import__("os").environ.get("KV2_PDB","5")), space="PSUM"))
            pv = _mp.enter_context(tc.tile_pool(name="pv", bufs=int(__import__("os").environ.get("KV2_PVB","2")), space="PSUM"))
            pm = _mp.enter_context(tc.tile_pool(name="pm", bufs=1, space="PSUM"))

            # ---- PE clock warmup: dummy DR matmuls on memset tiles run
            # while the first real DMAs are in flight, so the p-state ramp
            # completes before the first real k-loop.
            dwu = cp.tile([128, 256], fp8, tag="dwu")
            nc.vector.memset(dwu[:], 0.0)
            dpd = pd.tile([128, 128], f32, tag="pd", name="warm_pd")
            dwv = dwu[:].rearrange("p (j c) -> p j c", j=2)
            if N_WARM:
                for i in range(N_WARM):
                    nc.tensor.matmul(dpd[:], dwv, dwv,
                                     start=(i == 0), stop=(i == N_WARM - 1),
                                     perf_mode=DR)

            def warm2():
                if not N_WARM2:
                    return
                # pv pool: its banks are untouched until the first P-block,
                # so this group cannot collide with the open pd k-loop groups
                dpd2 = pv.tile([128, 128], f32, tag="V", name="warm_pd2")
                for i in range(N_WARM2):
                    nc.tensor.matmul(dpd2[:], dwv, dwv,
                                     start=(i == 0), stop=(i == N_WARM2 - 1),
                                     perf_mode=DR)

            # ---- resident loads; emission order = DMA service order ----
            Ws = cp.tile([128, NM * nke * 2 * 128], fp8, tag="Ws")
            Wt = cp.tile([128, NM * NK2 * 2 * 128], fp8, tag="Wt")
            sh = cp.tile([128, nke * 2 * S], fp8, tag="sh")
            th = cp.tile([128, NK2 * 2 * S], fp8, tag="th")
            wstr = nke * 2 * 128       # Ws stride per m
            wttr = NK2 * 2 * 128       # Wt stride per m
            d_Wsf = d_Ws[:].rearrange("p m k j c -> p (m k j c)")
            d_Wtf = d_Wt[:].rearrange("p m k j c -> p (m k j c)")
            shv = sh[:].rearrange("p (k j s) -> p k j s", k=nke, j=2)
            thv = th[:].rearrange("p (k j s) -> p k j s", k=NK2, j=2)

            def load_w(m):
                nc.sync.dma_start(Ws[:, m * wstr:(m + 1) * wstr],
                                  d_Wsf[:, m * wstr:(m + 1) * wstr])
                nc.sync.dma_start(Wt[:, m * wttr:(m + 1) * wttr],
                                  d_Wtf[:, m * wttr:(m + 1) * wttr])

            def load_act(c):
                s0, s1 = c * 512, (c + 1) * 512
                nc.sync.dma_start(shv[:, :, :, s0:s1], d_sh[:, :, :, s0:s1])
                nc.sync.dma_start(thv[:, :, :, s0:s1], d_th[:, :, :, s0:s1])

            head_eng = nc.gpsimd if os.environ.get("KV2_HEADDMA", "sync") == "gps" \
                else nc.sync
            head_eng.dma_start(Ws[:, 0:wstr], d_Wsf[:, 0:wstr])
            head_eng.dma_start(shv[:, :, :, 0:512], d_sh[:, :, :, 0:512])
            head_eng.dma_start(Wt[:, 0:wttr], d_Wtf[:, 0:wttr])
            head_eng.dma_start(thv[:, :, :, 0:512], d_th[:, :, :, 0:512])
            wa_sb = cp.tile([128, NCHUNK * 256], bf16, tag="wa")

            def load_wa(c):
                nc.sync.dma_start(
                    wa_sb[:, c * 1024:(c + 1) * 1024],
                    d_wa[:].rearrange("p a b -> p (a b)")[:, c * 1024:(c + 1) * 1024])

            # straight weight stream: supply (1.46us/m-pair) stays ahead of
            # the steady k-loop demand (1.71us/m), so no mid-chunk stalls;
            # Bc and the chunk-1 acts land just before P(0)/k(1,0) need them
            for m in range(1, NM):
                load_w(m)
            Bc = cp.tile([128, NMP * 2 * 256], fp8, tag="Bc")
            nc.sync.dma_start(Bc[:], d_Bc[:].rearrange("p a b c -> p (a b c)"))
            load_act(1)
            load_act(2)
            load_act(3)
            for c in range(NC):
                load_wa(c)
            onesH = cp.tile([128, 1], bf16, tag="onesH")
            nc.sync.dma_start(onesH[:], d_onesH)

            acc128 = cp.tile([128, S], bf16, tag="acc128")
            nc.vector.memset(acc128[:], 0.0)
            fm = cp.tile([128, 32], f32, tag="fm")   # [fmse(16) | mb(16)]
            mbp = pm.tile([128, 16], f32, tag="pmisc")

            dTc0 = cp.tile([128, NM * 512], fp8, tag="dTc0")
            dTc1 = cp.tile([128, NM * 512], fp8, tag="dTc1")

            Wsv = Ws[:].rearrange("p (m k j c) -> p m k j c", m=NM, k=nke, j=2)
            Wtv = Wt[:].rearrange("p (m k j c) -> p m k j c", m=NM, k=NK2, j=2)

            CHUNKS = [(c * 512, 512) for c in range(NC)]
            NCH = len(CHUNKS)

            pds = {}

            def k_s(c, m):
                s0, w = CHUNKS[c]
                pd_t = pd.tile([128, w], f32, tag="pd", name=f"pd_{c}_{m}")
                pds[(c, m)] = pd_t
                for k2 in range(nke):
                    nc.tensor.matmul(pd_t[:], Wsv[:, m, k2],
                                     shv[:, k2, :, s0:s0 + w],
                                     start=(k2 == 0), stop=False, perf_mode=DR)

            def k_t(c, m, dTc):
                s0, w = CHUNKS[c]
                pd_t = pds.pop((c, m))
                for k2 in range(NK2):
                    nc.tensor.matmul(pd_t[:], Wtv[:, m, k2],
                                     thv[:, k2, :, s0:s0 + w],
                                     start=False, stop=(k2 == NK2 - 1),
                                     perf_mode=DR)
                nc.scalar.mul(dTc[:, m * 512:m * 512 + w], pd_t[:], DSC)
                sq = qp.tile([128, w], bf16, tag="sq", name=f"sq_{c}_{m}")
                nc.scalar.square(sq[:], pd_t[:])
                # bf16 accumulate: DVE 2x mode (all-16-bit packed SBUF ops);
                # ~0.3% noise on mean_base, far inside the 2e-2 budget
                with nc.allow_low_precision("bf16 mean-base accumulate"):
                    nc.vector.tensor_add(acc128[:, s0:s0 + w],
                                         acc128[:, s0:s0 + w], sq[:])

            def k_loop(c, m, dTc):
                k_s(c, m)
                k_t(c, m, dTc)

            ACT_RED_SUBS = tuple(int(x) for x in os.environ.get(
                "KV2_ACTRED", "0,1,2").split(",") if x != "")

            def consume_v(Vt, chunk, sub):
                # wacat already carries wsel * a * scale: one multiply + one
                # full-width reduce per 128-token chunk. Multiplies alternate
                # DVE / GpSimd and reduces DVE / ACT-accumulator, spreading
                # the final chunk's drain across three engines.
                prod = vp.tile([128, 256], f32, tag="prod",
                               name=f"prod_{chunk}")
                nc.vector.tensor_tensor(prod[:], Vt[:],
                                        wa_sb[:, chunk * 256:(chunk + 1) * 256],
                                        ALU.mult)
                if sub in ACT_RED_SUBS:
                    # ACT accumulator takes one reduce off DVE; the LAST
                    # sub's reduce stays on DVE so the tail's final hop
                    # avoids a cross-engine semaphore
                    nc.scalar.activation(
                        prod[:], prod[:], mybir.ActivationFunctionType.Copy,
                        accum_out=fm[:, chunk:chunk + 1])
                else:
                    nc.vector.tensor_reduce(fm[:, chunk:chunk + 1], prod[:],
                                            axis=AX.X, op=ALU.add)

            # P-block split: the head (m-pairs 0..5) depends only on dT8
            # tiles the ACT queue finished long ago; the rest (pairs 6, 7 +
            # consume) waits on the final dT8 copies and is emitted one
            # k-loop later so PE never idles on the ACT lag.
            def p_head(c, dTc):
                nsub = CHUNKS[c][1] // 128
                dv = dTc[:].rearrange("p (m s) -> p m s", m=NM)
                Bv = Bc[:].rearrange("p (a b c) -> p a b c", a=NMP, b=2)
                Vts = []
                for sub in range(nsub):
                    t0 = sub * 128
                    Vt = pv.tile([128, 256], f32, tag="V", name=f"V_{c}_{sub}")
                    Vts.append(Vt)
                    for m2 in range(NMP - 1):
                        nc.tensor.matmul(
                            Vt[:],
                            dv[:, 2 * m2:2 * m2 + 2, t0:t0 + 128],
                            Bv[:, m2],
                            start=(m2 == 0), stop=False,
                            perf_mode=DR)
                return Vts

            def p_rest(c, dTc, Vts):
                s0, w = CHUNKS[c]
                dv = dTc[:].rearrange("p (m s) -> p m s", m=NM)
                Bv = Bc[:].rearrange("p (a b c) -> p a b c", a=NMP, b=2)
                for sub in range(w // 128):
                    t0 = sub * 128
                    for m2 in (NMP - 1,):
                        nc.tensor.matmul(
                            Vts[sub][:],
                            dv[:, 2 * m2:2 * m2 + 2, t0:t0 + 128],
                            Bv[:, m2],
                            start=False, stop=(m2 == NMP - 1),
                            perf_mode=DR)
                    consume_v(Vts[sub], s0 // 128 + sub, sub)

            def p_block(c, dTc):
                p_rest(c, dTc, p_head(c, dTc))

            def mb_mms(c):
                # per-128-token-chunk ones-matmuls over this chunk's acc128
                s0, w = CHUNKS[c]
                for i in range(w // 128):
                    cc = s0 // 128 + i
                    nc.tensor.matmul(mbp[:, cc:cc + 1],
                                     acc128[:, cc * 128:(cc + 1) * 128],
                                     onesH[:], start=True, stop=True)

            # ---- software-pipelined main loop ----
            # chunk 0 runs the s-matmuls 3 m-tiles ahead of the t-matmuls so
            # PE need not wait for the th chunk-0 DMA; later chunks pair s/t.
            # P-matmuls of chunk c are emitted after k(c+1, m=0) so they never
            # wait on the in-flight ACT dT8 copies.
            dTcs = [dTc0, dTc1]
            SKEW = int(os.environ.get("KV2_SKEW", "3"))
            for m in range(NM + SKEW):
                if m < NM:
                    k_s(0, m)
                if m == SKEW - 1:
                    warm2()
                if m >= SKEW:
                    k_t(0, m - SKEW, dTcs[0])
            # P(0) before k(1,0): chunk-1 activations are still in flight
            # when chunk 0 ends, so the P-block fills that DMA wait. For
            # later chunks the acts are long resident and P runs after
            # k(c,0) to hide the dT8 ACT lag instead.
            if P0_FIRST:
                p_block(0, dTcs[0])
                mb_mms(0)
            pend = None
            for c in range(1, NCH):
                for m in range(NM):
                    k_loop(c, m, dTcs[c % 2])
                    if m == 0 and (c > 1 or not P0_FIRST):
                        pend = p_head(c - 1, dTcs[(c - 1) % 2])
                    if m == 1 and pend is not None:
                        p_rest(c - 1, dTcs[(c - 1) % 2], pend)
                        mb_mms(c - 1)
                        pend = None
            p_block(NCH - 1, dTcs[(NCH - 1) % 2])
            mb_mms(NCH - 1)
            nc.scalar.copy(fm[:, 16:32], mbp[:, 0:16])
            _mp.close()
            nc.sync.dma_start(d_fm, fm[:])

    nc.compile()
    return nc


def _get_program(db_nonzero: bool):
    key = bool(db_nonzero)
    if key not in _PROGRAM_CACHE:
        _PROGRAM_CACHE[key] = _build_program(key)
    return _PROGRAM_CACHE[key]


# ----------------------------------------------------------------------------
# host side
# ----------------------------------------------------------------------------

def _host_scan_all(tg_all, sg_all, mask_f, gumbel):
    """Method-A sampling scan, all cores vectorized. Exact argmax semantics.
    Returns (wsel[B,S,E] f32, wsum f64, t_counts[E] f64, s_counts[E] f64)."""
    f32 = np.float32
    p = tg_all.astype(f32).copy()
    wsel = np.zeros((B, S, E), f32)
    s_counts = np.zeros(E, np.float64)
    BIG = f32(1e4)
    iota = np.arange(E, dtype=f32)
    for k in range(K):
        z = np.log(p) + gumbel[k]
        m = z.max(-1, keepdims=True)
        ge = (z >= m).astype(f32)
        t = iota + BIG - BIG * ge
        idxf = t.min(-1, keepdims=True)
        oh = (iota == idxf).astype(f32)
        po = p * oh
        w = po.sum(-1)
        sg_k = (sg_all * oh).sum(-1)
        mw = mask_f * w
        wsel += mw[..., None] * oh
        s_counts += ((mask_f * sg_k)[..., None] * oh).astype(np.float64).sum(axis=(0, 1))
        if k < K - 1:
            pn = p + (ALPHA - 1.0) * po
            p = pn / pn.sum(-1, keepdims=True)
    t_counts = wsel.astype(np.float64).sum(axis=(0, 1))
    wsum = float(t_counts.sum())
    return wsel, wsum, t_counts, s_counts


def _host_method_b(tg, sg, temp_c):
    """Per-core method-B partials: (tkl, ent)."""
    f32 = np.float32
    tg = tg.astype(f32)
    sg = sg.astype(f32)
    sgT = sg / f32(temp_c)
    ltg = np.log(tg)
    lsg = np.log(sg)
    ent = (sg * lsg).sum(dtype=f32)
    mb2 = sgT.max(-1, keepdims=True)
    ex = np.exp(sgT - mb2)
    se = ex.sum(-1, keepdims=True, dtype=f32)
    lse = np.log(se) + mb2
    sum_tg = tg.sum(-1, keepdims=True, dtype=f32)
    tkl = (tg * (ltg - sgT)).sum(dtype=f32) + (lse * sum_tg).sum(dtype=f32)
    return tkl, ent


def _host_quad(a_s_all, a_t_all, wsel, B_s, B_t):
    """Sum over tokens/experts of wsel * quad / H (the LoRA Gram term)."""
    G_ss = np.einsum("ehr,ehq->erq", B_s, B_s)
    G_st = np.einsum("ehr,ehq->erq", B_s, B_t)
    G_tt = np.einsum("ehr,ehq->erq", B_t, B_t)
    acc = 0.0
    for e in range(E):
        q1 = ((a_s_all @ G_ss[e]) * a_s_all).sum(-1)
        q2 = ((a_s_all @ G_st[e]) * a_t_all).sum(-1)
        q3 = ((a_t_all @ G_tt[e]) * a_t_all).sum(-1)
        qe = (SCALE_S * SCALE_S) * q1 - (2.0 * SCALE_S * SCALE_T) * q2 \
            + (SCALE_T * SCALE_T) * q3
        acc += float((wsel[:, :, e].astype(np.float64) * qe).sum())
    return acc / H


def _prep_shared(inputs, db_nonzero):
    """Replicated (per-core identical) device arrays."""
    f32 = np.float32
    W_t = np.asarray(inputs["W_t"], f32)
    W_s = np.asarray(inputs["W_s"], f32)
    B_t = np.asarray(inputs["B_t"], f32)
    B_s = np.asarray(inputs["B_s"], f32)
    db = (np.asarray(inputs["b_s"], f32) - np.asarray(inputs["b_t"], f32))

    nke = NK2 + (1 if db_nonzero else 0)

    # W layout [p, m, k2, j, c] = WSC * W[m*128+c, k2*256+j*128+p]
    def w_host(W, k_tiles, bias=None):
        out = np.zeros((128, NM, k_tiles, 2, 128), F8)
        out[:, :, :NK2] = (WSC * W).astype(F8).reshape(
            NM, 128, NK2, 2, 128).transpose(4, 0, 2, 3, 1)
        if bias is not None and k_tiles > NK2:
            out[0, :, NK2, 0, :] = (WSC * bias).astype(F8).reshape(NM, 128)
        return np.ascontiguousarray(out)

    Ws8 = w_host(W_s, nke, db if db_nonzero else None)
    Wt8 = w_host(-W_t, NK2)   # negated: PSUM accumulation adds, d = base_s - base_t

    # Bc8 [p, m2, j, col] = BSC * [Bs_her | Bt_her][m2*256+j*128+p, col]
    Bs_her = B_s.transpose(1, 0, 2).reshape(H, E * R)
    Bt_her = B_t.transpose(1, 0, 2).reshape(H, E * R)
    B_cat = np.concatenate([Bs_her, Bt_her], axis=1)
    Bc8 = np.ascontiguousarray(
        (BSC * B_cat).astype(F8).reshape(NMP, 2, 128, 256).transpose(2, 0, 1, 3))

    onesH = np.ones((128, 1), BF16)

    shared = dict(Ws8=Ws8, Wt8=Wt8, Bc8=Bc8, onesH=onesH)
    return shared, nke


def _prep_core(sh, th, a_s, a_t, nke, wsel, db_nonzero):
    """Per-core device arrays (plus host-side wsel_e for the mb combine)."""
    f32 = np.float32

    # wacat[t, (half, e, r)] = wsel[t, e] * f_half * a_half[t, r]: folds the
    # expert selection weights and cross-term scales into the V consume.
    fs = f32(2.0 * SCALE_S / (H * VSC))
    ft = f32(-2.0 * SCALE_T / (H * VSC))
    wa_s = wsel[:, :, None] * (fs * a_s)[:, None, :]       # [S, E, R]
    wa_t = wsel[:, :, None] * (ft * a_t)[:, None, :]
    wa = np.concatenate([wa_s.reshape(S, E * R), wa_t.reshape(S, E * R)],
                        axis=1)                            # [S, 256]
    wacat = np.ascontiguousarray(
        wa.reshape(NCHUNK, 128, 256).transpose(1, 0, 2)).astype(BF16)

    # [p, k2, j, s] layout of x.T
    def xt_host(x, k_tiles, ones_tail=False):
        out = np.zeros((128, k_tiles, 2, S), F8)
        out[:, :NK2] = x.T.astype(F8).reshape(NK2, 2, 128, S).transpose(2, 0, 1, 3)
        if ones_tail and k_tiles > NK2:
            out[0, NK2, 0, :] = F8(1.0)
        return np.ascontiguousarray(out)

    sh8 = xt_host(sh, nke, ones_tail=(nke > NK2))
    th8 = xt_host(th, NK2)

    wsel_e = np.ascontiguousarray(
        wsel.sum(-1).reshape(NCHUNK, 128).T).astype(f32)   # [128, NCHUNK]
    dev = dict(sh8=sh8, th8=th8, wacat=wacat)
    return dev, wsel_e


def _combine(feat_parts, feat_quad, wsum, t_counts, s_counts, tkls, ents, temp_c):
    f32 = np.float32
    feat = np.sum(np.asarray(feat_parts, f32), dtype=f32) + f32(feat_quad)
    tc = np.asarray(t_counts, np.float64)
    sc = np.asarray(s_counts, np.float64)
    tkl = np.sum(np.asarray(tkls, f32), dtype=f32)
    ent = np.sum(np.asarray(ents, f32), dtype=f32)

    feat_loss = feat / max(wsum, 1e-8)
    t_avg = tc / tc.sum() + EPS
    s_avg = sc / sc.sum() + EPS
    t_avg = t_avg / t_avg.sum()
    s_avg = s_avg / s_avg.sum()
    coverage_kl = (t_avg * (np.log(t_avg) - np.log(s_avg))).sum() / E
    method_a_total = feat_loss + LAMBDA_COV * coverage_kl
    temp_kl = tkl / B
    entropy_loss = ent / (B * S)
    method_b_total = temp_kl + BETA_ENT * entropy_loss
    return np.array(
        [feat_loss, coverage_kl, method_a_total, temp_kl, entropy_loss,
         method_b_total, temp_c], f32)


def _host_all(inputs):
    """Host scan/method-B/quad for all cores + per-core device input maps."""
    f32 = np.float32
    db_nonzero = bool(
        np.any(np.asarray(inputs["b_s"], f32) != np.asarray(inputs["b_t"], f32)))
    temp = float(np.asarray(inputs["temperature"], f32))
    temp_c = float(np.clip(temp, TEMP_LO, TEMP_HI))

    u = np.asarray(inputs["uniform_noise"], f32)
    gumbel = -np.log(-np.log(u * (1.0 - 2e-7) + 1e-7)).astype(f32)
    mask_f = np.asarray(inputs["attention_mask"], f32)
    tg_all = np.asarray(inputs["teacher_gates"], f32)
    sg_all = np.asarray(inputs["student_gates"], f32)

    shared, nke = _prep_shared(inputs, db_nonzero)
    wsel_all, wsum, t_counts, s_counts = _host_scan_all(
        tg_all, sg_all, mask_f, gumbel)

    A_sT = np.ascontiguousarray(np.asarray(inputs["A_s"], f32).T)
    A_tT = np.ascontiguousarray(np.asarray(inputs["A_t"], f32).T)
    sh_all = np.asarray(inputs["student_hidden_states"], f32)
    th_all = np.asarray(inputs["teacher_hidden_states"], f32)

    in_maps = []
    tkls, ents, wsel_es = [], [], []
    a_s_all = np.empty((B, S, R), f32)
    a_t_all = np.empty((B, S, R), f32)
    for c in range(B):
        tkl, ent = _host_method_b(tg_all[c], sg_all[c], temp_c)
        tkls.append(tkl)
        ents.append(ent)
        a_s = sh_all[c] @ A_sT
        a_t = th_all[c] @ A_tT
        a_s_all[c] = a_s
        a_t_all[c] = a_t
        m = dict(shared)
        dev, wsel_e = _prep_core(sh_all[c], th_all[c], a_s, a_t, nke,
                                 wsel_all[c], db_nonzero)
        m.update(dev)
        in_maps.append(m)
        wsel_es.append(wsel_e)

    feat_quad = _host_quad(a_s_all, a_t_all, wsel_all,
                           np.asarray(inputs["B_s"], f32),
                           np.asarray(inputs["B_t"], f32))

    return dict(in_maps=in_maps, db_nonzero=db_nonzero, temp_c=temp_c,
                wsum=wsum, t_counts=t_counts, s_counts=s_counts,
                tkls=tkls, ents=ents, feat_quad=feat_quad, wsel_es=wsel_es)


def kernel(**inputs) -> np.ndarray:
    host = _host_all(inputs)
    nc = _get_program(host["db_nonzero"])

    from concourse.bass_utils import run_bass_kernel_spmd

    res = run_bass_kernel_spmd(nc, host["in_maps"], core_ids=list(range(B)))
    feat_parts = []
    for c in range(B):
        fm = np.asarray(res.results[c]["fm"], np.float32)   # [128, 32]
        fmse = fm[:, 0:16]
        mb = fm[:, 16:32]
        feat_parts.append(float(fmse.sum(dtype=np.float64))
                          + float((mb * host["wsel_es"][c]).sum(dtype=np.float64))
                          / (H * WSC * WSC))

    return _combine(feat_parts, host["feat_quad"], host["wsum"],
                    host["t_counts"], host["s_counts"], host["tkls"],
                    host["ents"], host["temp_c"])
